# revision 7
# baseline (speedup 1.0000x reference)
"""2-layer GCN (GCNConv x2 + ReLU) on 8 Trainium2 NeuronCores.

Contract: kernel(**inputs) takes FULL inputs (x [100000,64] f32,
edge_index [2,1600000] i32, W1 [64,64], b1 [64], W2 [64,32], b2 [32])
and returns the FULL output [100000, 32] f32.

Strategy (graph/data parallel, by-dst gather, bf16 compute):
  - Nodes sharded 8 ways by contiguous dst range (12500/core, padded to
    12544 = 98 blocks of 128). out = relu(dis * scatter_add_dst(g[src])
    + b) with g = (act @ W) * dis.
  - Self-loops are NOT in the edge stream: per dst block a single
    identity matmul injects sl = (act @ W) * dis^2 into the PSUM
    accumulator (lhsT=sl_block, rhs=I128), also opening each PSUM
    accumulation bank (start flag).
  - All tables/operands bf16 (PE 1 cycle/row vs 4 for fp32; DVE 2x_1p).
    Gather rows are 256B (dma_gather minimum): [64 real | 64 junk] for
    layer 1, [32 real | 96 junk] for layer 2.
  - Layer-1 dense is REPLICATED (full padded graph per core) from a
    host-staged bf16 transposed x; layer-2 dense is own-shard only,
    exported via 4 chunked AllGathers of the compact real-only [*,32]
    bf16 table (4x fewer bytes than padded rows), then expanded into
    256B gather rows with a strided DRAM-to-DRAM copy.
  - A separate tiny own-shard dense pass produces the self-loop tiles
    in SBUF (per-core input; no DRAM round trip).
  - Edge phase: host packs edges into 128-edge tiles grouped by
    (sweep of up to 8 dst blocks, src-quarter chunk, dst block) with
    per-(block,chunk) tile quotas = max over cores so all 8 cores run
    ONE program. dma_gather pulls 256B rows; one-hot built by is_equal
    on bf16 APs whose last dim is a real [1,2] pair (dl staged
    duplicated) to hit the DVE 2x_1p mode; PE matmul msg^T @ onehot
    accumulates feat-major PSUM per block; finalize multiplies dis[dst]
    and applies Relu+bias on the Activation engine.
"""

import sys

if "/opt/trn_rl_repo" not in sys.path:
    sys.path.insert(0, "/opt/trn_rl_repo")

import numpy as np
from ml_dtypes import bfloat16

N = 100000
IN = 64
HID = 64
OUT = 32
C = 8                  # cores
NPC = N // C           # 12500 real nodes per core
BLK = 128              # dst nodes per block / one-hot width
NBLK = 98              # blocks per core (12544 padded nodes)
NP = NBLK * BLK        # 12544 padded nodes per core
SWMAX = 8              # max blocks per sweep (2 PSUM banks)
DCH = 6                # dense-phase blocks per psum chunk (1 bank)
PADDL = 300.0          # dstlocal for pad slots (no one-hot match)
OH_GRP = 8             # tiles per chained one-hot build


def _quarters():
    """Node-quarters (in blocks) per core; chunk j gather table covers
    all 8 ranks' quarter-j rows and must stay < 32767 rows."""
    q = NBLK // 4
    qb = [q, q, q, NBLK - 3 * q]
    assert max(qb) * BLK * C < 32767
    return qb


def _sweeps():
    """[(n_blocks, quarter)] covering each quarter with <=SWMAX blocks."""
    out = []
    for j, nq in enumerate(_quarters()):
        left = nq
        while left > 0:
            take = min(SWMAX, left)
            out.append((take, j))
            left -= take
    return out


# ----------------------------------------------------------------------------
# Host-side packing
# ----------------------------------------------------------------------------

def _pack(edge_index):
    src = edge_index[0].astype(np.int64)
    dst = edge_index[1].astype(np.int64)

    # deg includes self-loops (reference adds them), >= 1
    deg = (np.bincount(dst, minlength=N) + 1).astype(np.float32)
    dis = (1.0 / np.sqrt(deg)).astype(np.float32)

    qb = _quarters()
    qrows = [b * BLK for b in qb]
    qbase = np.cumsum([0] + qrows[:-1])        # row base of quarter, padded
    trows = [C * r for r in qrows]             # gather-table rows per chunk

    # gather-table position of a source node (within its chunk's table)
    rank = src // NPC
    off = src % NPC
    chunk = np.searchsorted(qbase, off, side="right") - 1   # 0..3
    tidx = rank * np.asarray(qrows)[chunk] + (off - qbase[chunk])

    core = dst // NPC
    dloc = dst - core * NPC
    block = dloc // BLK
    dlb = dloc % BLK

    key = (core * NBLK + block) * 4 + chunk
    counts = np.bincount(key, minlength=C * NBLK * 4).reshape(C, NBLK, 4)
    quota = -(-counts.max(axis=0) // 128)  # [NBLK, 4]

    sweeps = _sweeps()
    nsw = len(sweeps)
    szs = [s[0] for s in sweeps]
    sweep_base = np.cumsum([0] + szs[:-1])
    sweep_of_block = np.repeat(np.arange(nsw), szs)
    lb_of_block = np.arange(NBLK) - sweep_base[sweep_of_block]

    sweep_goff = np.cumsum([0] + [4 * sz for sz in szs[:-1]])
    gid_of_bj = (sweep_goff[sweep_of_block][:, None]
                 + np.arange(4)[None, :] * np.array(szs)[sweep_of_block][:, None]
                 + lb_of_block[:, None])
    ngroups = 4 * NBLK
    gq = np.zeros(ngroups, np.int64)
    gq[gid_of_bj.reshape(-1)] = quota.reshape(-1)
    gbase = np.zeros_like(gq)
    np.cumsum(gq[:-1], out=gbase[1:])
    tiles_total = int(gq.sum())
    slots_total = tiles_total * 128

    g_sj = np.zeros((nsw, 4), np.int64)
    call_base = np.zeros((nsw, 4), np.int64)
    for s in range(nsw):
        b0 = sweep_base[s]
        for j in range(4):
            g_sj[s, j] = quota[b0:b0 + szs[s], j].sum()
    cb = np.zeros(nsw * 4, np.int64)
    np.cumsum(g_sj.reshape(-1)[:-1], out=cb[1:])
    call_base[:] = cb.reshape(nsw, 4)

    meta = dict(quota=quota, sweeps=sweeps, sweep_base=sweep_base,
                qb=qb, qrows=qrows, qbase=qbase, trows=trows,
                g_sj=g_sj, call_base=call_base, tiles_total=tiles_total,
                slots_total=slots_total)

    per_core = []
    for c in range(C):
        m = core == c
        gid = gid_of_bj[block[m], chunk[m]]
        order = np.argsort(gid, kind="stable")
        gid_s = gid[order]
        grp_start = np.searchsorted(gid_s, np.arange(ngroups))
        pos = np.arange(gid_s.size) - grp_start[gid_s]
        slot = gbase[gid_s] * 128 + pos
        assert (pos < gq[gid_s] * 128).all()

        idx_slots = np.zeros(slots_total, np.int16)
        dl_slots = np.full(slots_total, PADDL, np.float32)
        idx_slots[slot] = tidx[m][order].astype(np.int16)
        dl_slots[slot] = dlb[m][order].astype(np.float32)

        iw = idx_slots.reshape(-1, 16).T.copy()
        idxw = np.tile(iw, (8, 1))
        # dl duplicated x2 along a trailing dim so the one-hot in1 AP ends
        # with a real [1,2] packed pair (DVE 2x_1p requirement)
        dlt = dl_slots.reshape(-1, 128).T.astype(bfloat16)   # [128, tiles]
        dlw2 = np.repeat(dlt, 2, axis=1)                     # [128, 2*tiles]

        dis_own = np.ones(NP, np.float32)
        dis_own[:NPC] = dis[c * NPC:(c + 1) * NPC]
        disw = dis_own.reshape(NBLK, 128).T.astype(bfloat16)   # [128, NBLK]
        disqw = (dis_own * dis_own).reshape(NBLK, 128).T.astype(bfloat16)
        dist = np.tile(dis_own[None, :], (64, 1)).astype(bfloat16)  # [64, NP]

        per_core.append(dict(idxw=idxw, dlw2=dlw2, disw=disw, disqw=disqw,
                             dist=dist))

    # replicated-dense dis staging (same for all cores)
    dis_pad_full = np.ones(C * NP, np.float32)
    for c in range(C):
        dis_pad_full[c * NP:c * NP + NPC] = dis[c * NPC:(c + 1) * NPC]
    diswf = dis_pad_full.reshape(C * NBLK, 128).T.astype(bfloat16)

    return meta, per_core, dis, diswf


def _stage_inputs(x, W1, b1, W2, b2, meta, per_core, diswf):
    x = np.asarray(x, np.float32)
    W2p = np.concatenate([np.asarray(W2, np.float32),
                          np.zeros((HID, HID - OUT), np.float32)], axis=1)
    iota = np.tile(np.arange(BLK, dtype=np.float32), (128, 1)).astype(bfloat16)
    eye = np.eye(128, dtype=np.float32).astype(bfloat16)
    xTf = np.zeros((IN, C * NP), np.float32)
    for r in range(C):
        xTf[:, r * NP:r * NP + NPC] = x[r * NPC:(r + 1) * NPC].T
    xTf = xTf.astype(bfloat16)
    in_maps = []
    for c in range(C):
        pc = per_core[c]
        in_maps.append({
            "xTf": xTf,
            "xTo": xTf[:, c * NP:(c + 1) * NP].copy(),
            "diswf": diswf,
            "disw": pc["disw"],
            "disqw": pc["disqw"],
            "dist": pc["dist"],
            "idxw": pc["idxw"],
            "dlw2": pc["dlw2"],
            "iota": iota,
            "eye": eye,
            "W1": np.asarray(W1, np.float32).astype(bfloat16),
            "W2p": W2p.astype(bfloat16),
            "b1": np.asarray(b1, np.float32).reshape(HID, 1),
            "b2": np.asarray(b2, np.float32).reshape(OUT, 1),
        })
    return in_maps


def _program_schedule(meta):
    """Per sweep: (sl_flags, seq). sl_flags[lb] = (start, stop) for the
    self-loop matmul of local block lb (emitted FIRST, before all edge
    matmuls). seq[j] = [(cursor_in_call, local_block, start, stop)].
    Flags are at per-(sweep, psum-bank) granularity; the self-loop
    matmul of the first block in each bank always carries start."""
    quota, sweeps, sweep_base = meta["quota"], meta["sweeps"], meta["sweep_base"]
    sched = []
    for s, (nb, _q) in enumerate(sweeps):
        b0 = sweep_base[s]
        seq = []
        for j in range(4):
            cur = 0
            call = []
            for lb in range(nb):
                q = int(quota[b0 + lb, j])
                for r in range(q):
                    call.append([cur, lb, False, False])
                    cur += 1
            seq.append(call)
        sl_flags = [[False, False] for _ in range(nb)]
        nbank = (nb + 3) // 4
        for k in range(nbank):
            # first touch: self-loop matmul of the bank's first block
            sl_flags[4 * k][0] = True
            # last touch: last edge matmul in this bank, else last self-loop
            touch = [(j, i) for j in range(4) for i, e in enumerate(seq[j])
                     if e[1] // 4 == k]
            if touch:
                j1, i1 = touch[-1]
                seq[j1][i1][3] = True
            else:
                lb_last = min(4 * k + 3, nb - 1)
                sl_flags[lb_last][1] = True
        sched.append((sl_flags, seq))
    return sched


def _dense_chunks(nblocks):
    out = []
    left = nblocks
    while left > 0:
        out.append(min(DCH, left))
        left -= out[-1]
    return out


# ----------------------------------------------------------------------------
# Device program (identical on all 8 cores)
# ----------------------------------------------------------------------------

def _build(meta):
    from concourse import bacc, mybir, tile

    sweeps = meta["sweeps"]
    nsw = len(sweeps)
    sweep_base = meta["sweep_base"]
    qb, qrows, qbase, trows = (meta["qb"], meta["qrows"], meta["qbase"],
                               meta["trows"])
    g_sj = meta["g_sj"]
    call_base = meta["call_base"]
    tiles_total = meta["tiles_total"]
    slots_total = meta["slots_total"]
    sched = _program_schedule(meta)
    qblk_base = [int(b) // BLK for b in qbase]   # quarter base, in blocks
    f32 = mybir.dt.float32
    bf16 = mybir.dt.bfloat16

    nc = bacc.Bacc(num_devices=C)
    d_xTf = nc.dram_tensor("xTf", [IN, C * NP], bf16, kind="ExternalInput")
    d_xTo = nc.dram_tensor("xTo", [IN, NP], bf16, kind="ExternalInput")
    d_diswf = nc.dram_tensor("diswf", [128, C * NBLK], bf16,
                             kind="ExternalInput")
    d_disw = nc.dram_tensor("disw", [128, NBLK], bf16, kind="ExternalInput")
    d_disqw = nc.dram_tensor("disqw", [128, NBLK], bf16, kind="ExternalInput")
    d_dist = nc.dram_tensor("dist", [64, NP], bf16, kind="ExternalInput")
    d_idxw = nc.dram_tensor("idxw", [128, slots_total // 16], mybir.dt.int16,
                            kind="ExternalInput")
    d_dlw2 = nc.dram_tensor("dlw2", [128, 2 * tiles_total], bf16,
                            kind="ExternalInput")
    d_iota = nc.dram_tensor("iota", [128, BLK], bf16, kind="ExternalInput")
    d_eye = nc.dram_tensor("eye", [128, 128], bf16, kind="ExternalInput")
    d_W1 = nc.dram_tensor("W1", [IN, HID], bf16, kind="ExternalInput")
    d_W2p = nc.dram_tensor("W2p", [HID, HID], bf16, kind="ExternalInput")
    d_b1 = nc.dram_tensor("b1", [HID, 1], f32, kind="ExternalInput")
    d_b2 = nc.dram_tensor("b2", [OUT, 1], f32, kind="ExternalInput")
    d_out = nc.dram_tensor("outT", [OUT, NP], f32, kind="ExternalOutput")

    with tile.TileContext(nc) as tc:
        with (
            tc.tile_pool(name="persist", bufs=1) as pp,
            tc.tile_pool(name="dram", bufs=1, space="DRAM") as dp,
        ):
            t_dlw2 = pp.tile([128, 2 * tiles_total], bf16, tag="dlw2")
            t_idx = pp.tile([128, slots_total // 16], mybir.dt.int16,
                            tag="idx")
            t_iota = pp.tile([128, BLK], bf16, tag="iota")
            t_eye = pp.tile([128, 128], bf16, tag="eye")
            t_W1 = pp.tile([IN, HID], bf16, tag="W1")
            t_W2p = pp.tile([HID, HID], bf16, tag="W2p")
            t_b1 = pp.tile([HID, 1], f32, tag="b1")
            t_b2 = pp.tile([OUT, 1], f32, tag="b2")
            t_diswf = pp.tile([128, C * NBLK], bf16, tag="diswf")
            t_disw = pp.tile([128, NBLK], bf16, tag="disw")
            t_disqw = pp.tile([128, NBLK], bf16, tag="disqw")
            t_dist = pp.tile([64, NP], bf16, tag="dist")
            t_h1T = pp.tile([64, NP], bf16, tag="h1T")
            t_sl1 = pp.tile([128, NBLK * 64], bf16, tag="sl1")
            t_sl2 = pp.tile([128, NBLK * 32], bf16, tag="sl2")

            nc.sync.dma_start(out=t_dlw2[:], in_=d_dlw2[:])
            nc.sync.dma_start(out=t_idx[:], in_=d_idxw[:])
            nc.sync.dma_start(out=t_iota[:], in_=d_iota[:])
            nc.sync.dma_start(out=t_eye[:], in_=d_eye[:])
            nc.sync.dma_start(out=t_W1[:], in_=d_W1[:])
            nc.sync.dma_start(out=t_W2p[:], in_=d_W2p[:])
            nc.sync.dma_start(out=t_b1[:], in_=d_b1[:])
            nc.sync.dma_start(out=t_b2[:], in_=d_b2[:])
            nc.sync.dma_start(out=t_diswf[:], in_=d_diswf[:])
            nc.sync.dma_start(out=t_disw[:], in_=d_disw[:])
            nc.sync.dma_start(out=t_disqw[:], in_=d_disqw[:])
            nc.sync.dma_start(out=t_dist[:], in_=d_dist[:])

            # DRAM scratch: gather tables (256B rows) + compact L2 export
            gtab = [[dp.tile([trows[j], 128], bf16, name=f"gtab{L}_{j}",
                             tag=f"gtab{L}_{j}")
                     for j in range(4)] for L in range(2)]
            g2c = dp.tile([NP, 32], bf16, name="g2c", tag="g2c")
            g2cg = [dp.tile([trows[j] // 4, 128], bf16, name=f"g2cg{j}",
                            tag=f"g2cg{j}")
                    for j in range(4)]

            def dense_chunk(qp, sp, lhs_ap, scale_cols, W, nb, outs):
                """One psum chunk: nb block-matmuls, then for each
                (scale_col_ap, dst writer) in outs: evict psum*scale."""
                p = qp.tile([128, DCH * 64], f32, tag="p")
                for t in range(nb):
                    nc.tensor.matmul(
                        out=p[:, t * 64:(t + 1) * 64],
                        lhsT=lhs_ap[:, t * 128:(t + 1) * 128],
                        rhs=W[:],
                        start=(t == 0), stop=(t == nb - 1),
                    )
                pv = p[:].rearrange("p (t f) -> p t f", f=64)[:, :nb, :]
                for scale_ap, writer in outs:
                    ev = sp.tile([128, DCH * 64], bf16, tag="ev")
                    evv = ev[:].rearrange("p (t f) -> p t f", f=64)[:, :nb, :]
                    nc.vector.tensor_tensor(
                        out=evv, in0=pv,
                        in1=scale_ap.unsqueeze(2).to_broadcast([128, nb, 64]),
                        op=mybir.AluOpType.mult,
                    )
                    writer(evv)
                _ = scale_cols  # unused; kept for call-site clarity

            # ---- own-shard dense passes -> self-loop tiles in SBUF
            # (tiny: no DRAM traffic; per-core via xTo/disqw inputs)
            with (
                tc.tile_pool(name="dzos", bufs=2) as spo,
                tc.tile_pool(name="dzox", bufs=2) as xpo,
                tc.tile_pool(name="dzop", bufs=2, space="PSUM") as qpo,
            ):
                xo = xpo.tile([64, NP], bf16, tag="xo")
                nc.sync.dma_start(out=xo[:], in_=d_xTo[:])
                bb = 0
                for nb in _dense_chunks(NBLK):
                    def wr_sl1(evv, bb=bb, nb=nb):
                        nc.vector.tensor_copy(
                            out=t_sl1[:].rearrange("p (t f) -> p t f", f=64)
                            [:, bb:bb + nb, :],
                            in_=evv,
                        )
                    dense_chunk(
                        qpo, spo, xo[:, bb * 128:(bb + nb) * 128],
                        None, t_W1,
                        nb,
                        [(t_disw[:, bb:bb + nb], wr_sl1)],
                    )
                    bb += nb

            # ---- layer-1 dense, replicated over the full padded graph.
            # quarter-major so gather table j completes early.
            with (
                tc.tile_pool(name="dz1s", bufs=3) as sp1,
                tc.tile_pool(name="dz1x", bufs=2) as xp1,
                tc.tile_pool(name="dz1p", bufs=2, space="PSUM") as qp1,
            ):
                for j in range(4):
                    tabv = gtab[0][j][:].rearrange("(t p) f -> p t f", p=128)
                    for r in range(C):
                        xs = xp1.tile([64, max(qrows)], bf16, tag="xs")
                        nc.sync.dma_start(
                            out=xs[:, :qrows[j]],
                            in_=d_xTf[:, r * NP + int(qbase[j]):
                                      r * NP + int(qbase[j]) + qrows[j]],
                        )
                        bb = 0
                        for nb in _dense_chunks(qb[j]):
                            gcol = r * NBLK + qblk_base[j] + bb  # diswf col
                            trow = r * qb[j] + bb  # block-row in table j

                            def wr_tab(evv, tabv=tabv, trow=trow, nb=nb):
                                nc.scalar.dma_start(
                                    out=tabv[:, trow:trow + nb, 0:64],
                                    in_=evv,
                                )
                            dense_chunk(
                                qp1, sp1,
                                xs[:, bb * 128:(bb + nb) * 128],
                                None, t_W1,
                                nb,
                                [(t_diswf[:, gcol:gcol + nb], wr_tab)],
                            )
                            bb += nb

            # ---- interleaved: layer-1 edge + per-quarter layer-2 dense + CC
            gmax = int(g_sj.max())

            def sweep_pre(L, s, qp):
                nb, _q = sweeps[s]
                nf = 64 if L == 0 else OUT
                sl = t_sl1 if L == 0 else t_sl2
                slw = 64 if L == 0 else 32
                sl_flags, _seq = sched[s]
                ps = qp.tile([64, SWMAX * BLK], f32, tag="ps")
                # self-loop injection opens each bank's accumulation group
                for lb in range(nb):
                    b = sweep_base[s] + lb
                    nc.tensor.matmul(
                        out=ps[:nf, lb * BLK:(lb + 1) * BLK],
                        lhsT=sl[:, b * slw:b * slw + nf],
                        rhs=t_eye[:],
                        start=sl_flags[lb][0], stop=sl_flags[lb][1],
                    )
                return ps

            def sweep_chunks(L, s, ps, gp, op_, chunks):
                nf = 64 if L == 0 else OUT
                _slf, seq = sched[s]
                for j in chunks:
                    G = int(g_sj[s, j])
                    if G == 0:
                        continue
                    tb = int(call_base[s, j])
                    gb = gp.tile([128, gmax, 128], bf16, tag="gb")
                    nc.gpsimd.dma_gather(
                        out_ap=gb[:, :G, :],
                        in_ap=gtab[L][j][:, :],
                        idxs_ap=t_idx[:, tb * 8:tb * 8 + G * 8],
                        num_idxs=G * 128,
                        num_idxs_reg=G * 128,
                        elem_size=128,
                        single_packet=False,
                    )
                    todo = seq[j]
                    for g0 in range(0, len(todo), OH_GRP):
                        grp = todo[g0:g0 + OH_GRP]
                        ng = len(grp)
                        oh = op_.tile([128, OH_GRP, BLK], bf16, tag="oh")
                        dl0 = tb + grp[0][0]
                        nc.vector.tensor_tensor(
                            out=oh[:, :ng, :].rearrange(
                                "p g (a b) -> p g a b", b=2),
                            in0=t_iota[:].rearrange("p (a b) -> p a b", b=2)
                                .unsqueeze(1).to_broadcast([128, ng, 64, 2]),
                            in1=t_dlw2[:, 2 * dl0:2 * (dl0 + ng)]
                                .rearrange("p (g b) -> p g b", b=2)
                                .unsqueeze(2).to_broadcast([128, ng, 64, 2]),
                            op=mybir.AluOpType.is_equal,
                        )
                        for k, (cu, lb, fst, lst) in enumerate(grp):
                            nc.tensor.matmul(
                                out=ps[:nf, lb * BLK:(lb + 1) * BLK],
                                lhsT=gb[:, cu, 0:nf],
                                rhs=oh[:, k, :],
                                start=fst, stop=lst,
                            )
            def sweep_fin(L, s, ps, fp, sop):
                nb, _q = sweeps[s]
                bias = t_b1 if L == 0 else t_b2
                nf = 64 if L == 0 else OUT
                if L == 1:
                    ob = sop.tile([OUT, SWMAX * BLK], f32, tag="ob")
                for lb in range(nb):
                    gcol = (sweep_base[s] + lb) * BLK
                    ft = fp.tile([nf, BLK], f32, tag="ft")
                    nc.vector.tensor_tensor(
                        out=ft[:],
                        in0=ps[:nf, lb * BLK:(lb + 1) * BLK],
                        in1=t_dist[:nf, gcol:gcol + BLK],
                        op=mybir.AluOpType.mult,
                    )
                    dst_ap = (t_h1T[:, gcol:gcol + BLK] if L == 0
                              else ob[:, lb * BLK:(lb + 1) * BLK])
                    nc.scalar.activation(
                        out=dst_ap, in_=ft[:],
                        func=mybir.ActivationFunctionType.Relu,
                        bias=bias[:, :1], scale=1.0,
                    )
                if L == 1:
                    c0 = sweep_base[s] * BLK
                    nc.sync.dma_start(
                        out=d_out[:, c0:c0 + nb * BLK],
                        in_=ob[:, :nb * BLK],
                    )

            def edge_sweep(L, s, gp, op_, fp, qp, sop):
                ps = sweep_pre(L, s, qp)
                sweep_chunks(L, s, ps, gp, op_, (0, 1, 2, 3))
                sweep_fin(L, s, ps, fp, sop)

            g2v = g2c[:].rearrange("(t p) f -> p t f", p=128)
            with (
                tc.tile_pool(name="eg0", bufs=4) as gp0,
                tc.tile_pool(name="eo0", bufs=4) as op0,
                tc.tile_pool(name="ef0", bufs=4) as fp0,
                tc.tile_pool(name="ep0", bufs=3, space="PSUM") as qp0,
                tc.tile_pool(name="es0", bufs=2) as sop0,
                tc.tile_pool(name="dz2s", bufs=2) as sp2,
                tc.tile_pool(name="dz2p", bufs=2, space="PSUM") as qp2,
            ):
                for qq in range(4):
                    for s in range(nsw):
                        if sweeps[s][1] == qq:
                            edge_sweep(0, s, gp0, op0, fp0, qp0, sop0)
                    # layer-2 dense for this quarter's own nodes, then CC
                    bb = 0
                    for nb in _dense_chunks(qb[qq]):
                        bglob = qblk_base[qq] + bb

                        def wr_g2(evv, bglob=bglob, nb=nb):
                            nc.sync.dma_start(
                                out=g2v[:, bglob:bglob + nb, :],
                                in_=evv[:, :, 0:32],
                            )

                        def wr_sl2(evv, bglob=bglob, nb=nb):
                            nc.vector.tensor_copy(
                                out=t_sl2[:].rearrange(
                                    "p (t f) -> p t f", f=32)
                                [:, bglob:bglob + nb, :],
                                in_=evv[:, :, 0:32],
                            )
                        dense_chunk(
                            qp2, sp2,
                            t_h1T[:, bglob * 128:(bglob + nb) * 128],
                            None, t_W2p,
                            nb,
                            [(t_disw[:, bglob:bglob + nb], wr_g2),
                             (t_disw[:, bglob:bglob + nb], wr_sl2)],
                        )
                        bb += nb
                    nc.gpsimd.collective_compute(
                        "AllGather", mybir.AluOpType.bypass,
                        replica_groups=[list(range(C))],
                        ins=[g2c[int(qbase[qq]):int(qbase[qq]) + qrows[qq],
                                 :].opt()],
                        outs=[g2cg[qq][:].opt()],
                    )
                    # expand compact [rows,32] into 256B gather rows
                    nc.sync.dma_start(
                        out=gtab[1][qq][:, 0:32],
                        in_=g2cg[qq][:].rearrange("r (a f) -> (r a) f", a=4),
                    )

            # ---- layer-2 edge
            with (
                tc.tile_pool(name="eg1", bufs=4) as gp1,
                tc.tile_pool(name="eo1", bufs=4) as op1,
                tc.tile_pool(name="ef1", bufs=4) as fp1,
                tc.tile_pool(name="ep1", bufs=3, space="PSUM") as qp1b,
                tc.tile_pool(name="es1", bufs=2) as sop1,
            ):
                K = 3
                pss = {}
                for s in range(min(K, nsw)):
                    pss[s] = sweep_pre(1, s, qp1b)
                    sweep_chunks(1, s, pss[s], gp1, op1, (0, 1, 2))
                for s in range(nsw):
                    if s < K:
                        sweep_chunks(1, s, pss[s], gp1, op1, (3,))
                        sweep_fin(1, s, pss.pop(s), fp1, sop1)
                    else:
                        edge_sweep(1, s, gp1, op1, fp1, qp1b, sop1)

    nc.finalize()
    return nc


# ----------------------------------------------------------------------------
# Entry point
# ----------------------------------------------------------------------------

_CACHE = {}


def _prepare(x, edge_index, W1, b1, W2, b2):
    ei = np.asarray(edge_index, dtype=np.int64)
    key = (ei.shape, hash(ei[:, ::65537].tobytes()))
    if _CACHE.get("key") != key:
        meta, per_core, _dis, diswf = _pack(ei)
        nc = _build(meta)
        _CACHE.update(key=key, meta=meta, per_core=per_core, nc=nc,
                      diswf=diswf)
    in_maps = _stage_inputs(x, W1, b1, W2, b2, _CACHE["meta"],
                            _CACHE["per_core"], _CACHE["diswf"])
    return _CACHE["nc"], in_maps


def kernel(x, edge_index, W1, b1, W2, b2):
    from concourse.bass_utils import run_bass_kernel_spmd

    nc, in_maps = _prepare(x, edge_index, W1, b1, W2, b2)
    res = run_bass_kernel_spmd(nc, in_maps, core_ids=list(range(C)))
    outs = []
    for c in range(C):
        outs.append(res.results[c]["outT"][:, :NPC])
    return np.concatenate(outs, axis=1).T.astype(np.float32)


# ----------------------------------------------------------------------------
# Host-side emulation (fast validation of the packing; no HW)
# ----------------------------------------------------------------------------

def emulate(x, edge_index, W1, b1, W2, b2):
    x = np.asarray(x, np.float32)
    meta, per_core, dis, _diswf = _pack(np.asarray(edge_index, np.int64))
    sweeps, sweep_base = meta["sweeps"], meta["sweep_base"]
    qrows = meta["qrows"]
    g_sj, call_base = meta["g_sj"], meta["call_base"]
    sched = _program_schedule(meta)
    W2p = np.concatenate([np.asarray(W2, np.float32),
                          np.zeros((HID, HID - OUT), np.float32)], 1)
    out_full = np.zeros((N, OUT), np.float32)

    def run_layer(acts, W, bias, nf):
        gown = []
        sloop = []
        for c in range(C):
            disp = np.ones(NP, np.float32)
            disp[:NPC] = dis[c * NPC:(c + 1) * NPC]
            g = (acts[c] @ W) * disp[:, None]
            gown.append(g.astype(np.float32))
            sloop.append(g)
        qa = np.cumsum([0] + qrows[:-1])
        gtabs = [np.concatenate([gown[r][qa[j]:qa[j] + qrows[j]]
                                 for r in range(C)]) for j in range(4)]
        new_acts = []
        for c in range(C):
            pc = per_core[c]
            idxw = pc["idxw"]
            dlw2 = pc["dlw2"].astype(np.float32)
            disp = np.ones(NP, np.float32)
            disp[:NPC] = dis[c * NPC:(c + 1) * NPC]
            sT = sloop[c].T[:64].copy()  # self-loop injection
            for s in range(len(sweeps)):
                _slf, seq = sched[s]
                for j in range(4):
                    G = int(g_sj[s, j])
                    if G == 0:
                        continue
                    tb = int(call_base[s, j])
                    iw = idxw[:16, tb * 8:(tb + G) * 8]
                    idxs = iw.T.reshape(-1)
                    rows = gtabs[j][idxs]
                    for (cu, lb, _f, _l) in seq[j]:
                        t = tb + cu
                        msg = rows[cu * 128:(cu + 1) * 128]
                        dl = dlw2[:, 2 * t]
                        oh = (dl[:, None] ==
                              np.arange(BLK, dtype=np.float32)[None, :])
                        blkcol = (sweep_base[s] + lb) * BLK
                        sT[:, blkcol:blkcol + BLK] += msg.T @ oh
            act = np.maximum(sT[:nf] * disp[None, :] + bias.reshape(-1, 1),
                             0.0)
            aT = np.zeros((NP, 64), np.float32)
            aT[:, :nf] = act.T
            new_acts.append(aT)
        return new_acts

    acts = []
    for c in range(C):
        a = np.zeros((NP, 64), np.float32)
        a[:NPC] = x[c * NPC:(c + 1) * NPC]
        acts.append(a)
    acts = run_layer(acts, np.asarray(W1, np.float32),
                     np.asarray(b1, np.float32), 64)
    acts = run_layer(acts, W2p, np.asarray(b2, np.float32), OUT)
    for c in range(C):
        out_full[c * NPC:(c + 1) * NPC] = acts[c][:NPC, :OUT]
    return out_full


# revision 8
# speedup vs baseline: 1.0171x; 1.0171x over previous
"""2-layer GCN (GCNConv x2 + ReLU) on 8 Trainium2 NeuronCores.

Contract: kernel(**inputs) takes FULL inputs (x [100000,64] f32,
edge_index [2,1600000] i32, W1 [64,64], b1 [64], W2 [64,32], b2 [32])
and returns the FULL output [100000, 32] f32.

Strategy (graph/data parallel, by-dst gather, bf16 compute):
  - Nodes sharded 8 ways by contiguous dst range (12500/core, padded to
    12544 = 98 blocks of 128). out = relu(dis * scatter_add_dst(g[src])
    + b) with g = (act @ W) * dis.
  - Self-loops are NOT in the edge stream: per dst block a single
    identity matmul injects sl = (act @ W) * dis^2 into the PSUM
    accumulator (lhsT=sl_block, rhs=I128), also opening each PSUM
    accumulation bank (start flag).
  - All tables/operands bf16 (PE 1 cycle/row vs 4 for fp32; DVE 2x_1p).
    Gather rows are 256B (dma_gather minimum): [64 real | 64 junk] for
    layer 1, [32 real | 96 junk] for layer 2.
  - Layer-1 dense is REPLICATED (full padded graph per core) from a
    host-staged bf16 transposed x; layer-2 dense is own-shard only,
    exported via 4 chunked AllGathers of the compact real-only [*,32]
    bf16 table (4x fewer bytes than padded rows), then expanded into
    256B gather rows with a strided DRAM-to-DRAM copy.
  - A separate tiny own-shard dense pass produces the self-loop tiles
    in SBUF (per-core input; no DRAM round trip).
  - Edge phase: host packs edges into 128-edge tiles grouped by
    (sweep of up to 8 dst blocks, src-quarter chunk, dst block) with
    per-(block,chunk) tile quotas = max over cores so all 8 cores run
    ONE program. dma_gather pulls 256B rows; one-hot built by is_equal
    on bf16 APs whose last dim is a real [1,2] pair (dl staged
    duplicated) to hit the DVE 2x_1p mode; PE matmul msg^T @ onehot
    accumulates feat-major PSUM per block; finalize multiplies dis[dst]
    and applies Relu+bias on the Activation engine.
"""

import sys

if "/opt/trn_rl_repo" not in sys.path:
    sys.path.insert(0, "/opt/trn_rl_repo")

import numpy as np
from ml_dtypes import bfloat16

N = 100000
IN = 64
HID = 64
OUT = 32
C = 8                  # cores
NPC = N // C           # 12500 real nodes per core
BLK = 128              # dst nodes per block / one-hot width
NBLK = 98              # blocks per core (12544 padded nodes)
NP = NBLK * BLK        # 12544 padded nodes per core
SWMAX = 8              # max blocks per sweep (2 PSUM banks)
DCH = 6                # dense-phase blocks per psum chunk (1 bank)
PADDL = 300.0          # dstlocal for pad slots (no one-hot match)
OH_GRP = 8             # tiles per chained one-hot build


def _quarters():
    """Node-quarters (in blocks) per core; chunk j gather table covers
    all 8 ranks' quarter-j rows and must stay < 32767 rows."""
    q = NBLK // 4
    qb = [q, q, q, NBLK - 3 * q]
    assert max(qb) * BLK * C < 32767
    return qb


def _sweeps():
    """[(n_blocks, quarter)] covering each quarter with <=SWMAX blocks."""
    out = []
    for j, nq in enumerate(_quarters()):
        left = nq
        while left > 0:
            take = min(SWMAX, left)
            out.append((take, j))
            left -= take
    return out


# ----------------------------------------------------------------------------
# Host-side packing
# ----------------------------------------------------------------------------

def _pack(edge_index):
    src = edge_index[0].astype(np.int64)
    dst = edge_index[1].astype(np.int64)

    # deg includes self-loops (reference adds them), >= 1
    deg = (np.bincount(dst, minlength=N) + 1).astype(np.float32)
    dis = (1.0 / np.sqrt(deg)).astype(np.float32)

    qb = _quarters()
    qrows = [b * BLK for b in qb]
    qbase = np.cumsum([0] + qrows[:-1])        # row base of quarter, padded
    trows = [C * r for r in qrows]             # gather-table rows per chunk

    # gather-table position of a source node (within its chunk's table)
    rank = src // NPC
    off = src % NPC
    chunk = np.searchsorted(qbase, off, side="right") - 1   # 0..3
    tidx = rank * np.asarray(qrows)[chunk] + (off - qbase[chunk])

    core = dst // NPC
    dloc = dst - core * NPC
    block = dloc // BLK
    dlb = dloc % BLK

    key = (core * NBLK + block) * 4 + chunk
    counts = np.bincount(key, minlength=C * NBLK * 4).reshape(C, NBLK, 4)
    quota = -(-counts.max(axis=0) // 128)  # [NBLK, 4]

    sweeps = _sweeps()
    nsw = len(sweeps)
    szs = [s[0] for s in sweeps]
    sweep_base = np.cumsum([0] + szs[:-1])
    sweep_of_block = np.repeat(np.arange(nsw), szs)
    lb_of_block = np.arange(NBLK) - sweep_base[sweep_of_block]

    sweep_goff = np.cumsum([0] + [4 * sz for sz in szs[:-1]])
    gid_of_bj = (sweep_goff[sweep_of_block][:, None]
                 + np.arange(4)[None, :] * np.array(szs)[sweep_of_block][:, None]
                 + lb_of_block[:, None])
    ngroups = 4 * NBLK
    gq = np.zeros(ngroups, np.int64)
    gq[gid_of_bj.reshape(-1)] = quota.reshape(-1)
    gbase = np.zeros_like(gq)
    np.cumsum(gq[:-1], out=gbase[1:])
    tiles_total = int(gq.sum())
    slots_total = tiles_total * 128

    g_sj = np.zeros((nsw, 4), np.int64)
    call_base = np.zeros((nsw, 4), np.int64)
    for s in range(nsw):
        b0 = sweep_base[s]
        for j in range(4):
            g_sj[s, j] = quota[b0:b0 + szs[s], j].sum()
    cb = np.zeros(nsw * 4, np.int64)
    np.cumsum(g_sj.reshape(-1)[:-1], out=cb[1:])
    call_base[:] = cb.reshape(nsw, 4)

    meta = dict(quota=quota, sweeps=sweeps, sweep_base=sweep_base,
                qb=qb, qrows=qrows, qbase=qbase, trows=trows,
                g_sj=g_sj, call_base=call_base, tiles_total=tiles_total,
                slots_total=slots_total)

    per_core = []
    for c in range(C):
        m = core == c
        gid = gid_of_bj[block[m], chunk[m]]
        order = np.argsort(gid, kind="stable")
        gid_s = gid[order]
        grp_start = np.searchsorted(gid_s, np.arange(ngroups))
        pos = np.arange(gid_s.size) - grp_start[gid_s]
        slot = gbase[gid_s] * 128 + pos
        assert (pos < gq[gid_s] * 128).all()

        idx_slots = np.zeros(slots_total, np.int16)
        dl_slots = np.full(slots_total, PADDL, np.float32)
        idx_slots[slot] = tidx[m][order].astype(np.int16)
        dl_slots[slot] = dlb[m][order].astype(np.float32)

        iw = idx_slots.reshape(-1, 16).T.copy()
        idxw = np.tile(iw, (8, 1))
        # dl duplicated x2 along a trailing dim so the one-hot in1 AP ends
        # with a real [1,2] packed pair (DVE 2x_1p requirement)
        dlt = dl_slots.reshape(-1, 128).T.astype(bfloat16)   # [128, tiles]
        dlw2 = np.repeat(dlt, 2, axis=1)                     # [128, 2*tiles]

        dis_own = np.ones(NP, np.float32)
        dis_own[:NPC] = dis[c * NPC:(c + 1) * NPC]
        disw = dis_own.reshape(NBLK, 128).T.astype(bfloat16)   # [128, NBLK]
        disqw = (dis_own * dis_own).reshape(NBLK, 128).T.astype(bfloat16)
        dist = np.tile(dis_own[None, :], (64, 1)).astype(bfloat16)  # [64, NP]

        per_core.append(dict(idxw=idxw, dlw2=dlw2, disw=disw, disqw=disqw,
                             dist=dist))

    # replicated-dense dis staging (same for all cores)
    dis_pad_full = np.ones(C * NP, np.float32)
    for c in range(C):
        dis_pad_full[c * NP:c * NP + NPC] = dis[c * NPC:(c + 1) * NPC]
    diswf = dis_pad_full.reshape(C * NBLK, 128).T.astype(bfloat16)

    return meta, per_core, dis, diswf


def _stage_inputs(x, W1, b1, W2, b2, meta, per_core, diswf):
    x = np.asarray(x, np.float32)
    W2p = np.concatenate([np.asarray(W2, np.float32),
                          np.zeros((HID, HID - OUT), np.float32)], axis=1)
    iota = np.tile(np.arange(BLK, dtype=np.float32), (128, 1)).astype(bfloat16)
    eye = np.eye(128, dtype=np.float32).astype(bfloat16)
    xTf = np.zeros((IN, C * NP), np.float32)
    for r in range(C):
        xTf[:, r * NP:r * NP + NPC] = x[r * NPC:(r + 1) * NPC].T
    xTf = xTf.astype(bfloat16)
    in_maps = []
    for c in range(C):
        pc = per_core[c]
        in_maps.append({
            "xTf": xTf,
            "xTo": xTf[:, c * NP:(c + 1) * NP].copy(),
            "diswf": diswf,
            "disw": pc["disw"],
            "disqw": pc["disqw"],
            "dist": pc["dist"],
            "idxw": pc["idxw"],
            "dlw2": pc["dlw2"],
            "iota": iota,
            "eye": eye,
            "W1": np.asarray(W1, np.float32).astype(bfloat16),
            "W2p": W2p.astype(bfloat16),
            "b1": np.asarray(b1, np.float32).reshape(HID, 1),
            "b2": np.asarray(b2, np.float32).reshape(OUT, 1),
        })
    return in_maps


def _program_schedule(meta):
    """Per sweep: (sl_flags, seq). sl_flags[lb] = (start, stop) for the
    self-loop matmul of local block lb (emitted FIRST, before all edge
    matmuls). seq[j] = [(cursor_in_call, local_block, start, stop)].
    Flags are at per-(sweep, psum-bank) granularity; the self-loop
    matmul of the first block in each bank always carries start."""
    quota, sweeps, sweep_base = meta["quota"], meta["sweeps"], meta["sweep_base"]
    sched = []
    for s, (nb, _q) in enumerate(sweeps):
        b0 = sweep_base[s]
        seq = []
        for j in range(4):
            cur = 0
            call = []
            for lb in range(nb):
                q = int(quota[b0 + lb, j])
                for r in range(q):
                    call.append([cur, lb, False, False])
                    cur += 1
            seq.append(call)
        sl_flags = [[False, False] for _ in range(nb)]
        nbank = (nb + 3) // 4
        for k in range(nbank):
            # first touch: self-loop matmul of the bank's first block
            sl_flags[4 * k][0] = True
            # last touch: last edge matmul in this bank, else last self-loop
            touch = [(j, i) for j in range(4) for i, e in enumerate(seq[j])
                     if e[1] // 4 == k]
            if touch:
                j1, i1 = touch[-1]
                seq[j1][i1][3] = True
            else:
                lb_last = min(4 * k + 3, nb - 1)
                sl_flags[lb_last][1] = True
        sched.append((sl_flags, seq))
    return sched


def _dense_chunks(nblocks):
    out = []
    left = nblocks
    while left > 0:
        out.append(min(DCH, left))
        left -= out[-1]
    return out


# ----------------------------------------------------------------------------
# Device program (identical on all 8 cores)
# ----------------------------------------------------------------------------

def _build(meta):
    from concourse import bacc, mybir, tile

    sweeps = meta["sweeps"]
    nsw = len(sweeps)
    sweep_base = meta["sweep_base"]
    qb, qrows, qbase, trows = (meta["qb"], meta["qrows"], meta["qbase"],
                               meta["trows"])
    g_sj = meta["g_sj"]
    call_base = meta["call_base"]
    tiles_total = meta["tiles_total"]
    slots_total = meta["slots_total"]
    sched = _program_schedule(meta)
    qblk_base = [int(b) // BLK for b in qbase]   # quarter base, in blocks
    f32 = mybir.dt.float32
    bf16 = mybir.dt.bfloat16

    nc = bacc.Bacc(num_devices=C)
    d_xTf = nc.dram_tensor("xTf", [IN, C * NP], bf16, kind="ExternalInput")
    d_xTo = nc.dram_tensor("xTo", [IN, NP], bf16, kind="ExternalInput")
    d_diswf = nc.dram_tensor("diswf", [128, C * NBLK], bf16,
                             kind="ExternalInput")
    d_disw = nc.dram_tensor("disw", [128, NBLK], bf16, kind="ExternalInput")
    d_disqw = nc.dram_tensor("disqw", [128, NBLK], bf16, kind="ExternalInput")
    d_dist = nc.dram_tensor("dist", [64, NP], bf16, kind="ExternalInput")
    d_idxw = nc.dram_tensor("idxw", [128, slots_total // 16], mybir.dt.int16,
                            kind="ExternalInput")
    d_dlw2 = nc.dram_tensor("dlw2", [128, 2 * tiles_total], bf16,
                            kind="ExternalInput")
    d_iota = nc.dram_tensor("iota", [128, BLK], bf16, kind="ExternalInput")
    d_eye = nc.dram_tensor("eye", [128, 128], bf16, kind="ExternalInput")
    d_W1 = nc.dram_tensor("W1", [IN, HID], bf16, kind="ExternalInput")
    d_W2p = nc.dram_tensor("W2p", [HID, HID], bf16, kind="ExternalInput")
    d_b1 = nc.dram_tensor("b1", [HID, 1], f32, kind="ExternalInput")
    d_b2 = nc.dram_tensor("b2", [OUT, 1], f32, kind="ExternalInput")
    d_out = nc.dram_tensor("outT", [OUT, NP], f32, kind="ExternalOutput")

    with tile.TileContext(nc) as tc:
        with (
            tc.tile_pool(name="persist", bufs=1) as pp,
            tc.tile_pool(name="dram", bufs=1, space="DRAM") as dp,
        ):
            t_dlw2 = pp.tile([128, 2 * tiles_total], bf16, tag="dlw2")
            t_idx = pp.tile([128, slots_total // 16], mybir.dt.int16,
                            tag="idx")
            t_iota = pp.tile([128, BLK], bf16, tag="iota")
            t_eye = pp.tile([128, 128], bf16, tag="eye")
            t_W1 = pp.tile([IN, HID], bf16, tag="W1")
            t_W2p = pp.tile([HID, HID], bf16, tag="W2p")
            t_b1 = pp.tile([HID, 1], f32, tag="b1")
            t_b2 = pp.tile([OUT, 1], f32, tag="b2")
            t_diswf = pp.tile([128, C * NBLK], bf16, tag="diswf")
            t_disw = pp.tile([128, NBLK], bf16, tag="disw")
            t_disqw = pp.tile([128, NBLK], bf16, tag="disqw")
            t_dist = pp.tile([64, NP], bf16, tag="dist")
            t_h1T = pp.tile([64, NP], bf16, tag="h1T")
            t_sl1 = pp.tile([128, NBLK * 64], bf16, tag="sl1")
            t_sl2 = pp.tile([128, NBLK * 32], bf16, tag="sl2")

            nc.sync.dma_start(out=t_dlw2[:], in_=d_dlw2[:])
            nc.sync.dma_start(out=t_idx[:], in_=d_idxw[:])
            nc.sync.dma_start(out=t_iota[:], in_=d_iota[:])
            nc.sync.dma_start(out=t_eye[:], in_=d_eye[:])
            nc.sync.dma_start(out=t_W1[:], in_=d_W1[:])
            nc.sync.dma_start(out=t_W2p[:], in_=d_W2p[:])
            nc.sync.dma_start(out=t_b1[:], in_=d_b1[:])
            nc.sync.dma_start(out=t_b2[:], in_=d_b2[:])
            nc.sync.dma_start(out=t_diswf[:], in_=d_diswf[:])
            nc.sync.dma_start(out=t_disw[:], in_=d_disw[:])
            nc.sync.dma_start(out=t_disqw[:], in_=d_disqw[:])
            nc.sync.dma_start(out=t_dist[:], in_=d_dist[:])

            # DRAM scratch: gather tables (256B rows) + compact L2 export
            gtab = [[dp.tile([trows[j], 128], bf16, name=f"gtab{L}_{j}",
                             tag=f"gtab{L}_{j}")
                     for j in range(4)] for L in range(2)]
            g2c = dp.tile([NP, 32], bf16, name="g2c", tag="g2c")
            g2cg = [dp.tile([trows[j] // 4, 128], bf16, name=f"g2cg{j}",
                            tag=f"g2cg{j}")
                    for j in range(4)]

            def dense_chunk(qp, sp, lhs_ap, scale_cols, W, nb, outs):
                """One psum chunk: nb block-matmuls, then for each
                (scale_col_ap, dst writer) in outs: evict psum*scale."""
                p = qp.tile([128, DCH * 64], f32, tag="p")
                for t in range(nb):
                    nc.tensor.matmul(
                        out=p[:, t * 64:(t + 1) * 64],
                        lhsT=lhs_ap[:, t * 128:(t + 1) * 128],
                        rhs=W[:],
                        start=(t == 0), stop=(t == nb - 1),
                    )
                pv = p[:].rearrange("p (t f) -> p t f", f=64)[:, :nb, :]
                for scale_ap, writer in outs:
                    ev = sp.tile([128, DCH * 64], bf16, tag="ev")
                    evv = ev[:].rearrange("p (t f) -> p t f", f=64)[:, :nb, :]
                    nc.vector.tensor_tensor(
                        out=evv, in0=pv,
                        in1=scale_ap.unsqueeze(2).to_broadcast([128, nb, 64]),
                        op=mybir.AluOpType.mult,
                    )
                    writer(evv)
                _ = scale_cols  # unused; kept for call-site clarity

            # ---- own-shard dense passes -> self-loop tiles in SBUF
            # (tiny: no DRAM traffic; per-core via xTo/disqw inputs)
            with (
                tc.tile_pool(name="dzos", bufs=2) as spo,
                tc.tile_pool(name="dzox", bufs=2) as xpo,
                tc.tile_pool(name="dzop", bufs=2, space="PSUM") as qpo,
            ):
                xo = xpo.tile([64, NP], bf16, tag="xo")
                nc.sync.dma_start(out=xo[:], in_=d_xTo[:])
                bb = 0
                for nb in _dense_chunks(NBLK):
                    def wr_sl1(evv, bb=bb, nb=nb):
                        nc.vector.tensor_copy(
                            out=t_sl1[:].rearrange("p (t f) -> p t f", f=64)
                            [:, bb:bb + nb, :],
                            in_=evv,
                        )
                    dense_chunk(
                        qpo, spo, xo[:, bb * 128:(bb + nb) * 128],
                        None, t_W1,
                        nb,
                        [(t_disw[:, bb:bb + nb], wr_sl1)],
                    )
                    bb += nb

            # ---- layer-1 dense, replicated over the full padded graph.
            # quarter-major so gather table j completes early.
            with (
                tc.tile_pool(name="dz1s", bufs=3) as sp1,
                tc.tile_pool(name="dz1x", bufs=2) as xp1,
                tc.tile_pool(name="dz1p", bufs=2, space="PSUM") as qp1,
            ):
                for j in range(4):
                    tabv = gtab[0][j][:].rearrange("(t p) f -> p t f", p=128)
                    for r in range(C):
                        xs = xp1.tile([64, max(qrows)], bf16, tag="xs")
                        nc.sync.dma_start(
                            out=xs[:, :qrows[j]],
                            in_=d_xTf[:, r * NP + int(qbase[j]):
                                      r * NP + int(qbase[j]) + qrows[j]],
                        )
                        bb = 0
                        for nb in _dense_chunks(qb[j]):
                            gcol = r * NBLK + qblk_base[j] + bb  # diswf col
                            trow = r * qb[j] + bb  # block-row in table j

                            def wr_tab(evv, tabv=tabv, trow=trow, nb=nb):
                                nc.scalar.dma_start(
                                    out=tabv[:, trow:trow + nb, 0:64],
                                    in_=evv,
                                )
                            dense_chunk(
                                qp1, sp1,
                                xs[:, bb * 128:(bb + nb) * 128],
                                None, t_W1,
                                nb,
                                [(t_diswf[:, gcol:gcol + nb], wr_tab)],
                            )
                            bb += nb

            # ---- interleaved: layer-1 edge + per-quarter layer-2 dense + CC
            gmax = int(g_sj.max())

            def sweep_pre(L, s, qp):
                nb, _q = sweeps[s]
                nf = 64 if L == 0 else OUT
                sl = t_sl1 if L == 0 else t_sl2
                slw = 64 if L == 0 else 32
                sl_flags, _seq = sched[s]
                ps = qp.tile([64, SWMAX * BLK], f32, tag="ps")
                # self-loop injection opens each bank's accumulation group
                for lb in range(nb):
                    b = sweep_base[s] + lb
                    nc.tensor.matmul(
                        out=ps[:nf, lb * BLK:(lb + 1) * BLK],
                        lhsT=sl[:, b * slw:b * slw + nf],
                        rhs=t_eye[:],
                        start=sl_flags[lb][0], stop=sl_flags[lb][1],
                    )
                return ps

            def sweep_chunks(L, s, ps, gp, op_, chunks):
                nf = 64 if L == 0 else OUT
                _slf, seq = sched[s]
                for j in chunks:
                    G = int(g_sj[s, j])
                    if G == 0:
                        continue
                    tb = int(call_base[s, j])
                    gb = gp.tile([128, gmax, 128], bf16, tag="gb")
                    nc.gpsimd.dma_gather(
                        out_ap=gb[:, :G, :],
                        in_ap=gtab[L][j][:, :],
                        idxs_ap=t_idx[:, tb * 8:tb * 8 + G * 8],
                        num_idxs=G * 128,
                        num_idxs_reg=G * 128,
                        elem_size=128,
                        single_packet=False,
                    )
                    todo = seq[j]
                    for g0 in range(0, len(todo), OH_GRP):
                        grp = todo[g0:g0 + OH_GRP]
                        ng = len(grp)
                        oh = op_.tile([128, OH_GRP, BLK], bf16, tag="oh")
                        dl0 = tb + grp[0][0]
                        nc.vector.tensor_tensor(
                            out=oh[:, :ng, :].rearrange(
                                "p g (a b) -> p g a b", b=2),
                            in0=t_iota[:].rearrange("p (a b) -> p a b", b=2)
                                .unsqueeze(1).to_broadcast([128, ng, 64, 2]),
                            in1=t_dlw2[:, 2 * dl0:2 * (dl0 + ng)]
                                .rearrange("p (g b) -> p g b", b=2)
                                .unsqueeze(2).to_broadcast([128, ng, 64, 2]),
                            op=mybir.AluOpType.is_equal,
                        )
                        for k, (cu, lb, fst, lst) in enumerate(grp):
                            nc.tensor.matmul(
                                out=ps[:nf, lb * BLK:(lb + 1) * BLK],
                                lhsT=gb[:, cu, 0:nf],
                                rhs=oh[:, k, :],
                                start=fst, stop=lst,
                            )
            def sweep_fin(L, s, ps, fp, sop):
                nb, _q = sweeps[s]
                bias = t_b1 if L == 0 else t_b2
                nf = 64 if L == 0 else OUT
                if L == 1:
                    ob = sop.tile([OUT, SWMAX * BLK], f32, tag="ob")
                for lb in range(nb):
                    gcol = (sweep_base[s] + lb) * BLK
                    ft = fp.tile([nf, BLK], f32, tag="ft")
                    nc.vector.tensor_tensor(
                        out=ft[:],
                        in0=ps[:nf, lb * BLK:(lb + 1) * BLK],
                        in1=t_dist[:nf, gcol:gcol + BLK],
                        op=mybir.AluOpType.mult,
                    )
                    dst_ap = (t_h1T[:, gcol:gcol + BLK] if L == 0
                              else ob[:, lb * BLK:(lb + 1) * BLK])
                    nc.scalar.activation(
                        out=dst_ap, in_=ft[:],
                        func=mybir.ActivationFunctionType.Relu,
                        bias=bias[:, :1], scale=1.0,
                    )
                if L == 1:
                    c0 = sweep_base[s] * BLK
                    nc.sync.dma_start(
                        out=d_out[:, c0:c0 + nb * BLK],
                        in_=ob[:, :nb * BLK],
                    )

            def edge_sweep(L, s, gp, op_, fp, qp, sop):
                ps = sweep_pre(L, s, qp)
                sweep_chunks(L, s, ps, gp, op_, (0, 1, 2, 3))
                sweep_fin(L, s, ps, fp, sop)

            g2v = g2c[:].rearrange("(t p) f -> p t f", p=128)
            with (
                tc.tile_pool(name="eg0", bufs=4) as gp0,
                tc.tile_pool(name="eo0", bufs=4) as op0,
                tc.tile_pool(name="ef0", bufs=4) as fp0,
                tc.tile_pool(name="ep0", bufs=3, space="PSUM") as qp0,
                tc.tile_pool(name="es0", bufs=2) as sop0,
                tc.tile_pool(name="dz2s", bufs=2) as sp2,
                tc.tile_pool(name="dz2p", bufs=2, space="PSUM") as qp2,
            ):
                for qq in range(4):
                    for s in range(nsw):
                        if sweeps[s][1] == qq:
                            edge_sweep(0, s, gp0, op0, fp0, qp0, sop0)
                    # layer-2 dense for this quarter's own nodes, then CC
                    bb = 0
                    for nb in _dense_chunks(qb[qq]):
                        bglob = qblk_base[qq] + bb

                        def wr_g2(evv, bglob=bglob, nb=nb):
                            nc.sync.dma_start(
                                out=g2v[:, bglob:bglob + nb, :],
                                in_=evv[:, :, 0:32],
                            )

                        def wr_sl2(evv, bglob=bglob, nb=nb):
                            nc.vector.tensor_copy(
                                out=t_sl2[:].rearrange(
                                    "p (t f) -> p t f", f=32)
                                [:, bglob:bglob + nb, :],
                                in_=evv[:, :, 0:32],
                            )
                        dense_chunk(
                            qp2, sp2,
                            t_h1T[:, bglob * 128:(bglob + nb) * 128],
                            None, t_W2p,
                            nb,
                            [(t_disw[:, bglob:bglob + nb], wr_g2),
                             (t_disw[:, bglob:bglob + nb], wr_sl2)],
                        )
                        bb += nb
                    nc.gpsimd.collective_compute(
                        "AllGather", mybir.AluOpType.bypass,
                        replica_groups=[list(range(C))],
                        ins=[g2c[int(qbase[qq]):int(qbase[qq]) + qrows[qq],
                                 :].opt()],
                        outs=[g2cg[qq][:].opt()],
                    )
                    # expand compact [rows,32] into 256B gather rows
                    nc.sync.dma_start(
                        out=gtab[1][qq][:, 0:32],
                        in_=g2cg[qq][:].rearrange("r (a f) -> (r a) f", a=4),
                    )

            # ---- layer-2 edge
            with (
                tc.tile_pool(name="eg1", bufs=4) as gp1,
                tc.tile_pool(name="eo1", bufs=4) as op1,
                tc.tile_pool(name="ef1", bufs=4) as fp1,
                tc.tile_pool(name="ep1", bufs=4, space="PSUM") as qp1b,
                tc.tile_pool(name="es1", bufs=2) as sop1,
            ):
                K = 4
                pss = {}
                for s in range(min(K, nsw)):
                    pss[s] = sweep_pre(1, s, qp1b)
                    sweep_chunks(1, s, pss[s], gp1, op1, (0, 1, 2))
                for s in range(nsw):
                    if s < K:
                        sweep_chunks(1, s, pss[s], gp1, op1, (3,))
                        sweep_fin(1, s, pss.pop(s), fp1, sop1)
                    else:
                        edge_sweep(1, s, gp1, op1, fp1, qp1b, sop1)

    nc.finalize()
    return nc


# ----------------------------------------------------------------------------
# Entry point
# ----------------------------------------------------------------------------

_CACHE = {}


def _prepare(x, edge_index, W1, b1, W2, b2):
    ei = np.asarray(edge_index, dtype=np.int64)
    key = (ei.shape, hash(ei[:, ::65537].tobytes()))
    if _CACHE.get("key") != key:
        meta, per_core, _dis, diswf = _pack(ei)
        nc = _build(meta)
        _CACHE.update(key=key, meta=meta, per_core=per_core, nc=nc,
                      diswf=diswf)
    in_maps = _stage_inputs(x, W1, b1, W2, b2, _CACHE["meta"],
                            _CACHE["per_core"], _CACHE["diswf"])
    return _CACHE["nc"], in_maps


def kernel(x, edge_index, W1, b1, W2, b2):
    from concourse.bass_utils import run_bass_kernel_spmd

    nc, in_maps = _prepare(x, edge_index, W1, b1, W2, b2)
    res = run_bass_kernel_spmd(nc, in_maps, core_ids=list(range(C)))
    outs = []
    for c in range(C):
        outs.append(res.results[c]["outT"][:, :NPC])
    return np.concatenate(outs, axis=1).T.astype(np.float32)


# ----------------------------------------------------------------------------
# Host-side emulation (fast validation of the packing; no HW)
# ----------------------------------------------------------------------------

def emulate(x, edge_index, W1, b1, W2, b2):
    x = np.asarray(x, np.float32)
    meta, per_core, dis, _diswf = _pack(np.asarray(edge_index, np.int64))
    sweeps, sweep_base = meta["sweeps"], meta["sweep_base"]
    qrows = meta["qrows"]
    g_sj, call_base = meta["g_sj"], meta["call_base"]
    sched = _program_schedule(meta)
    W2p = np.concatenate([np.asarray(W2, np.float32),
                          np.zeros((HID, HID - OUT), np.float32)], 1)
    out_full = np.zeros((N, OUT), np.float32)

    def run_layer(acts, W, bias, nf):
        gown = []
        sloop = []
        for c in range(C):
            disp = np.ones(NP, np.float32)
            disp[:NPC] = dis[c * NPC:(c + 1) * NPC]
            g = (acts[c] @ W) * disp[:, None]
            gown.append(g.astype(np.float32))
            sloop.append(g)
        qa = np.cumsum([0] + qrows[:-1])
        gtabs = [np.concatenate([gown[r][qa[j]:qa[j] + qrows[j]]
                                 for r in range(C)]) for j in range(4)]
        new_acts = []
        for c in range(C):
            pc = per_core[c]
            idxw = pc["idxw"]
            dlw2 = pc["dlw2"].astype(np.float32)
            disp = np.ones(NP, np.float32)
            disp[:NPC] = dis[c * NPC:(c + 1) * NPC]
            sT = sloop[c].T[:64].copy()  # self-loop injection
            for s in range(len(sweeps)):
                _slf, seq = sched[s]
                for j in range(4):
                    G = int(g_sj[s, j])
                    if G == 0:
                        continue
                    tb = int(call_base[s, j])
                    iw = idxw[:16, tb * 8:(tb + G) * 8]
                    idxs = iw.T.reshape(-1)
                    rows = gtabs[j][idxs]
                    for (cu, lb, _f, _l) in seq[j]:
                        t = tb + cu
                        msg = rows[cu * 128:(cu + 1) * 128]
                        dl = dlw2[:, 2 * t]
                        oh = (dl[:, None] ==
                              np.arange(BLK, dtype=np.float32)[None, :])
                        blkcol = (sweep_base[s] + lb) * BLK
                        sT[:, blkcol:blkcol + BLK] += msg.T @ oh
            act = np.maximum(sT[:nf] * disp[None, :] + bias.reshape(-1, 1),
                             0.0)
            aT = np.zeros((NP, 64), np.float32)
            aT[:, :nf] = act.T
            new_acts.append(aT)
        return new_acts

    acts = []
    for c in range(C):
        a = np.zeros((NP, 64), np.float32)
        a[:NPC] = x[c * NPC:(c + 1) * NPC]
        acts.append(a)
    acts = run_layer(acts, np.asarray(W1, np.float32),
                     np.asarray(b1, np.float32), 64)
    acts = run_layer(acts, W2p, np.asarray(b2, np.float32), OUT)
    for c in range(C):
        out_full[c * NPC:(c + 1) * NPC] = acts[c][:NPC, :OUT]
    return out_full


# revision 9
# speedup vs baseline: 1.0351x; 1.0176x over previous
"""2-layer GCN (GCNConv x2 + ReLU) on 8 Trainium2 NeuronCores.

Contract: kernel(**inputs) takes FULL inputs (x [100000,64] f32,
edge_index [2,1600000] i32, W1 [64,64], b1 [64], W2 [64,32], b2 [32])
and returns the FULL output [100000, 32] f32.

Strategy (graph/data parallel, by-dst gather, bf16 compute):
  - Nodes sharded 8 ways by contiguous dst range (12500/core, padded to
    12544 = 98 blocks of 128). out = relu(dis * scatter_add_dst(g[src])
    + b) with g = (act @ W) * dis.
  - Self-loops are NOT in the edge stream: per dst block a single
    identity matmul injects sl = (act @ W) * dis^2 into the PSUM
    accumulator (lhsT=sl_block, rhs=I128), also opening each PSUM
    accumulation bank (start flag).
  - All tables/operands bf16 (PE 1 cycle/row vs 4 for fp32; DVE 2x_1p).
    Gather rows are 256B (dma_gather minimum): [64 real | 64 junk] for
    layer 1, [32 real | 96 junk] for layer 2.
  - Layer-1 dense is REPLICATED (full padded graph per core) from a
    host-staged bf16 transposed x; layer-2 dense is own-shard only,
    exported via 4 chunked AllGathers of the compact real-only [*,32]
    bf16 table (4x fewer bytes than padded rows), then expanded into
    256B gather rows with a strided DRAM-to-DRAM copy.
  - A separate tiny own-shard dense pass produces the self-loop tiles
    in SBUF (per-core input; no DRAM round trip).
  - Edge phase: host packs edges into 128-edge tiles grouped by
    (sweep of up to 8 dst blocks, src-quarter chunk, dst block) with
    per-(block,chunk) tile quotas = max over cores so all 8 cores run
    ONE program. dma_gather pulls 256B rows; one-hot built by is_equal
    on bf16 APs whose last dim is a real [1,2] pair (dl staged
    duplicated) to hit the DVE 2x_1p mode; PE matmul msg^T @ onehot
    accumulates feat-major PSUM per block; finalize multiplies dis[dst]
    and applies Relu+bias on the Activation engine.
"""

import sys

if "/opt/trn_rl_repo" not in sys.path:
    sys.path.insert(0, "/opt/trn_rl_repo")

import numpy as np
from ml_dtypes import bfloat16

N = 100000
IN = 64
HID = 64
OUT = 32
C = 8                  # cores
NPC = N // C           # 12500 real nodes per core
BLK = 128              # dst nodes per block / one-hot width
NBLK = 98              # blocks per core (12544 padded nodes)
NP = NBLK * BLK        # 12544 padded nodes per core
SWMAX = 8              # max blocks per sweep (2 PSUM banks)
DCH = 6                # dense-phase blocks per psum chunk (1 bank)
PADDL = 300.0          # dstlocal for pad slots (no one-hot match)
OH_GRP = 8             # tiles per chained one-hot build


def _quarters():
    """Node-quarters (in blocks) per core; chunk j gather table covers
    all 8 ranks' quarter-j rows and must stay < 32767 rows."""
    q = NBLK // 4
    qb = [q, q, q, NBLK - 3 * q]
    assert max(qb) * BLK * C < 32767
    return qb


def _sweeps():
    """[(n_blocks, quarter)] covering each quarter with <=SWMAX blocks."""
    out = []
    for j, nq in enumerate(_quarters()):
        left = nq
        while left > 0:
            take = min(SWMAX, left)
            out.append((take, j))
            left -= take
    return out


# ----------------------------------------------------------------------------
# Host-side packing
# ----------------------------------------------------------------------------

def _pack(edge_index):
    src = edge_index[0].astype(np.int64)
    dst = edge_index[1].astype(np.int64)

    # deg includes self-loops (reference adds them), >= 1
    deg = (np.bincount(dst, minlength=N) + 1).astype(np.float32)
    dis = (1.0 / np.sqrt(deg)).astype(np.float32)

    qb = _quarters()
    qrows = [b * BLK for b in qb]
    qbase = np.cumsum([0] + qrows[:-1])        # row base of quarter, padded
    trows = [C * r for r in qrows]             # gather-table rows per chunk

    # gather-table position of a source node (within its chunk's table)
    rank = src // NPC
    off = src % NPC
    chunk = np.searchsorted(qbase, off, side="right") - 1   # 0..3
    tidx = rank * np.asarray(qrows)[chunk] + (off - qbase[chunk])

    core = dst // NPC
    dloc = dst - core * NPC
    block = dloc // BLK
    dlb = dloc % BLK

    key = (core * NBLK + block) * 4 + chunk
    counts = np.bincount(key, minlength=C * NBLK * 4).reshape(C, NBLK, 4)
    quota = -(-counts.max(axis=0) // 128)  # [NBLK, 4]

    sweeps = _sweeps()
    nsw = len(sweeps)
    szs = [s[0] for s in sweeps]
    sweep_base = np.cumsum([0] + szs[:-1])
    sweep_of_block = np.repeat(np.arange(nsw), szs)
    lb_of_block = np.arange(NBLK) - sweep_base[sweep_of_block]

    sweep_goff = np.cumsum([0] + [4 * sz for sz in szs[:-1]])
    gid_of_bj = (sweep_goff[sweep_of_block][:, None]
                 + np.arange(4)[None, :] * np.array(szs)[sweep_of_block][:, None]
                 + lb_of_block[:, None])
    ngroups = 4 * NBLK
    gq = np.zeros(ngroups, np.int64)
    gq[gid_of_bj.reshape(-1)] = quota.reshape(-1)
    gbase = np.zeros_like(gq)
    np.cumsum(gq[:-1], out=gbase[1:])
    tiles_total = int(gq.sum())
    slots_total = tiles_total * 128

    g_sj = np.zeros((nsw, 4), np.int64)
    call_base = np.zeros((nsw, 4), np.int64)
    for s in range(nsw):
        b0 = sweep_base[s]
        for j in range(4):
            g_sj[s, j] = quota[b0:b0 + szs[s], j].sum()
    cb = np.zeros(nsw * 4, np.int64)
    np.cumsum(g_sj.reshape(-1)[:-1], out=cb[1:])
    call_base[:] = cb.reshape(nsw, 4)

    meta = dict(quota=quota, sweeps=sweeps, sweep_base=sweep_base,
                qb=qb, qrows=qrows, qbase=qbase, trows=trows,
                g_sj=g_sj, call_base=call_base, tiles_total=tiles_total,
                slots_total=slots_total)

    per_core = []
    for c in range(C):
        m = core == c
        gid = gid_of_bj[block[m], chunk[m]]
        order = np.argsort(gid, kind="stable")
        gid_s = gid[order]
        grp_start = np.searchsorted(gid_s, np.arange(ngroups))
        pos = np.arange(gid_s.size) - grp_start[gid_s]
        slot = gbase[gid_s] * 128 + pos
        assert (pos < gq[gid_s] * 128).all()

        idx_slots = np.zeros(slots_total, np.int16)
        dl_slots = np.full(slots_total, PADDL, np.float32)
        idx_slots[slot] = tidx[m][order].astype(np.int16)
        dl_slots[slot] = dlb[m][order].astype(np.float32)

        iw = idx_slots.reshape(-1, 16).T.copy()
        idxw = np.tile(iw, (8, 1))
        # dl duplicated x2 along a trailing dim so the one-hot in1 AP ends
        # with a real [1,2] packed pair (DVE 2x_1p requirement)
        dlt = dl_slots.reshape(-1, 128).T.astype(bfloat16)   # [128, tiles]
        dlw2 = np.repeat(dlt, 2, axis=1)                     # [128, 2*tiles]

        dis_own = np.ones(NP, np.float32)
        dis_own[:NPC] = dis[c * NPC:(c + 1) * NPC]
        disw = dis_own.reshape(NBLK, 128).T.astype(bfloat16)   # [128, NBLK]
        disqw = (dis_own * dis_own).reshape(NBLK, 128).T.astype(bfloat16)
        dist = np.tile(dis_own[None, :], (64, 1)).astype(bfloat16)  # [64, NP]

        per_core.append(dict(idxw=idxw, dlw2=dlw2, disw=disw, disqw=disqw,
                             dist=dist))

    # replicated-dense dis staging (same for all cores)
    dis_pad_full = np.ones(C * NP, np.float32)
    for c in range(C):
        dis_pad_full[c * NP:c * NP + NPC] = dis[c * NPC:(c + 1) * NPC]
    diswf = dis_pad_full.reshape(C * NBLK, 128).T.astype(bfloat16)

    return meta, per_core, dis, diswf


def _stage_inputs(x, W1, b1, W2, b2, meta, per_core, diswf):
    x = np.asarray(x, np.float32)
    W2p = np.concatenate([np.asarray(W2, np.float32),
                          np.zeros((HID, HID - OUT), np.float32)], axis=1)
    iota = np.tile(np.arange(BLK, dtype=np.float32), (128, 1)).astype(bfloat16)
    eye = np.eye(128, dtype=np.float32).astype(bfloat16)
    xTf = np.zeros((IN, C * NP), np.float32)
    for r in range(C):
        xTf[:, r * NP:r * NP + NPC] = x[r * NPC:(r + 1) * NPC].T
    xTf = xTf.astype(bfloat16)
    in_maps = []
    for c in range(C):
        pc = per_core[c]
        in_maps.append({
            "xTf": xTf,
            "xTo": xTf[:, c * NP:(c + 1) * NP].copy(),
            "diswf": diswf,
            "disw": pc["disw"],
            "disqw": pc["disqw"],
            "dist": pc["dist"],
            "idxw": pc["idxw"],
            "dlw2": pc["dlw2"],
            "iota": iota,
            "eye": eye,
            "W1": np.asarray(W1, np.float32).astype(bfloat16),
            "W2p": W2p.astype(bfloat16),
            "b1": np.asarray(b1, np.float32).reshape(HID, 1),
            "b2": np.asarray(b2, np.float32).reshape(OUT, 1),
        })
    return in_maps


def _program_schedule(meta):
    """Per sweep: (sl_flags, seq). sl_flags[lb] = (start, stop) for the
    self-loop matmul of local block lb (emitted FIRST, before all edge
    matmuls). seq[j] = [(cursor_in_call, local_block, start, stop)].
    Flags are at per-(sweep, psum-bank) granularity; the self-loop
    matmul of the first block in each bank always carries start."""
    quota, sweeps, sweep_base = meta["quota"], meta["sweeps"], meta["sweep_base"]
    sched = []
    for s, (nb, _q) in enumerate(sweeps):
        b0 = sweep_base[s]
        seq = []
        for j in range(4):
            cur = 0
            call = []
            for lb in range(nb):
                q = int(quota[b0 + lb, j])
                for r in range(q):
                    call.append([cur, lb, False, False])
                    cur += 1
            seq.append(call)
        sl_flags = [[False, False] for _ in range(nb)]
        nbank = (nb + 3) // 4
        for k in range(nbank):
            # first touch: self-loop matmul of the bank's first block
            sl_flags[4 * k][0] = True
            # last touch: last edge matmul in this bank, else last self-loop
            touch = [(j, i) for j in range(4) for i, e in enumerate(seq[j])
                     if e[1] // 4 == k]
            if touch:
                j1, i1 = touch[-1]
                seq[j1][i1][3] = True
            else:
                lb_last = min(4 * k + 3, nb - 1)
                sl_flags[lb_last][1] = True
        sched.append((sl_flags, seq))
    return sched


def _dense_chunks(nblocks):
    out = []
    left = nblocks
    while left > 0:
        out.append(min(DCH, left))
        left -= out[-1]
    return out


# ----------------------------------------------------------------------------
# Device program (identical on all 8 cores)
# ----------------------------------------------------------------------------

def _build(meta):
    from concourse import bacc, mybir, tile

    sweeps = meta["sweeps"]
    nsw = len(sweeps)
    sweep_base = meta["sweep_base"]
    qb, qrows, qbase, trows = (meta["qb"], meta["qrows"], meta["qbase"],
                               meta["trows"])
    g_sj = meta["g_sj"]
    call_base = meta["call_base"]
    tiles_total = meta["tiles_total"]
    slots_total = meta["slots_total"]
    sched = _program_schedule(meta)
    qblk_base = [int(b) // BLK for b in qbase]   # quarter base, in blocks
    f32 = mybir.dt.float32
    bf16 = mybir.dt.bfloat16

    nc = bacc.Bacc(num_devices=C)
    d_xTf = nc.dram_tensor("xTf", [IN, C * NP], bf16, kind="ExternalInput")
    d_xTo = nc.dram_tensor("xTo", [IN, NP], bf16, kind="ExternalInput")
    d_diswf = nc.dram_tensor("diswf", [128, C * NBLK], bf16,
                             kind="ExternalInput")
    d_disw = nc.dram_tensor("disw", [128, NBLK], bf16, kind="ExternalInput")
    d_disqw = nc.dram_tensor("disqw", [128, NBLK], bf16, kind="ExternalInput")
    d_dist = nc.dram_tensor("dist", [64, NP], bf16, kind="ExternalInput")
    d_idxw = nc.dram_tensor("idxw", [128, slots_total // 16], mybir.dt.int16,
                            kind="ExternalInput")
    d_dlw2 = nc.dram_tensor("dlw2", [128, 2 * tiles_total], bf16,
                            kind="ExternalInput")
    d_iota = nc.dram_tensor("iota", [128, BLK], bf16, kind="ExternalInput")
    d_eye = nc.dram_tensor("eye", [128, 128], bf16, kind="ExternalInput")
    d_W1 = nc.dram_tensor("W1", [IN, HID], bf16, kind="ExternalInput")
    d_W2p = nc.dram_tensor("W2p", [HID, HID], bf16, kind="ExternalInput")
    d_b1 = nc.dram_tensor("b1", [HID, 1], f32, kind="ExternalInput")
    d_b2 = nc.dram_tensor("b2", [OUT, 1], f32, kind="ExternalInput")
    d_out = nc.dram_tensor("outT", [OUT, NP], f32, kind="ExternalOutput")

    with tile.TileContext(nc) as tc:
        with (
            tc.tile_pool(name="persist", bufs=1) as pp,
            tc.tile_pool(name="dram", bufs=1, space="DRAM") as dp,
        ):
            t_dlw2 = pp.tile([128, 2 * tiles_total], bf16, tag="dlw2")
            t_idx = pp.tile([128, slots_total // 16], mybir.dt.int16,
                            tag="idx")
            t_iota = pp.tile([128, BLK], bf16, tag="iota")
            t_eye = pp.tile([128, 128], bf16, tag="eye")
            t_W1 = pp.tile([IN, HID], bf16, tag="W1")
            t_W2p = pp.tile([HID, HID], bf16, tag="W2p")
            t_b1 = pp.tile([HID, 1], f32, tag="b1")
            t_b2 = pp.tile([OUT, 1], f32, tag="b2")
            t_diswf = pp.tile([128, C * NBLK], bf16, tag="diswf")
            t_disw = pp.tile([128, NBLK], bf16, tag="disw")
            t_disqw = pp.tile([128, NBLK], bf16, tag="disqw")
            t_dist = pp.tile([64, NP], bf16, tag="dist")
            t_h1T = pp.tile([64, NP], bf16, tag="h1T")
            t_sl1 = pp.tile([128, NBLK * 64], bf16, tag="sl1")
            t_sl2 = pp.tile([128, NBLK * 32], bf16, tag="sl2")

            # dense-phase prerequisites first; bulky edge-phase-only
            # tensors (idx/dl/dist) are deferred below so the dense
            # pipeline starts ~20us earlier
            nc.sync.dma_start(out=t_W1[:], in_=d_W1[:])
            nc.sync.dma_start(out=t_W2p[:], in_=d_W2p[:])
            nc.sync.dma_start(out=t_b1[:], in_=d_b1[:])
            nc.sync.dma_start(out=t_b2[:], in_=d_b2[:])
            nc.sync.dma_start(out=t_diswf[:], in_=d_diswf[:])
            nc.sync.dma_start(out=t_disw[:], in_=d_disw[:])
            nc.sync.dma_start(out=t_disqw[:], in_=d_disqw[:])
            nc.sync.dma_start(out=t_eye[:], in_=d_eye[:])

            # DRAM scratch: gather tables (256B rows) + compact L2 export
            gtab = [[dp.tile([trows[j], 128], bf16, name=f"gtab{L}_{j}",
                             tag=f"gtab{L}_{j}")
                     for j in range(4)] for L in range(2)]
            g2c = dp.tile([NP, 32], bf16, name="g2c", tag="g2c")
            g2cg = [dp.tile([trows[j] // 4, 128], bf16, name=f"g2cg{j}",
                            tag=f"g2cg{j}")
                    for j in range(4)]

            def dense_chunk(qp, sp, lhs_ap, scale_cols, W, nb, outs):
                """One psum chunk: nb block-matmuls, then for each
                (scale_col_ap, dst writer) in outs: evict psum*scale."""
                p = qp.tile([128, DCH * 64], f32, tag="p")
                for t in range(nb):
                    nc.tensor.matmul(
                        out=p[:, t * 64:(t + 1) * 64],
                        lhsT=lhs_ap[:, t * 128:(t + 1) * 128],
                        rhs=W[:],
                        start=(t == 0), stop=(t == nb - 1),
                    )
                pv = p[:].rearrange("p (t f) -> p t f", f=64)[:, :nb, :]
                for scale_ap, writer in outs:
                    ev = sp.tile([128, DCH * 64], bf16, tag="ev")
                    evv = ev[:].rearrange("p (t f) -> p t f", f=64)[:, :nb, :]
                    nc.vector.tensor_tensor(
                        out=evv, in0=pv,
                        in1=scale_ap.unsqueeze(2).to_broadcast([128, nb, 64]),
                        op=mybir.AluOpType.mult,
                    )
                    writer(evv)
                _ = scale_cols  # unused; kept for call-site clarity

            # ---- own-shard dense passes -> self-loop tiles in SBUF
            # (tiny: no DRAM traffic; per-core via xTo/disqw inputs)
            with (
                tc.tile_pool(name="dzos", bufs=2) as spo,
                tc.tile_pool(name="dzox", bufs=2) as xpo,
                tc.tile_pool(name="dzop", bufs=2, space="PSUM") as qpo,
            ):
                xo = xpo.tile([64, NP], bf16, tag="xo")
                nc.sync.dma_start(out=xo[:], in_=d_xTo[:])
                bb = 0
                for nb in _dense_chunks(NBLK):
                    def wr_sl1(evv, bb=bb, nb=nb):
                        nc.vector.tensor_copy(
                            out=t_sl1[:].rearrange("p (t f) -> p t f", f=64)
                            [:, bb:bb + nb, :],
                            in_=evv,
                        )
                    dense_chunk(
                        qpo, spo, xo[:, bb * 128:(bb + nb) * 128],
                        None, t_W1,
                        nb,
                        [(t_disw[:, bb:bb + nb], wr_sl1)],
                    )
                    bb += nb

            nc.sync.dma_start(out=t_dlw2[:], in_=d_dlw2[:])
            nc.sync.dma_start(out=t_idx[:], in_=d_idxw[:])
            nc.sync.dma_start(out=t_iota[:], in_=d_iota[:])
            nc.sync.dma_start(out=t_dist[:], in_=d_dist[:])

            # ---- layer-1 dense, replicated over the full padded graph.
            # quarter-major so gather table j completes early.
            with (
                tc.tile_pool(name="dz1s", bufs=3) as sp1,
                tc.tile_pool(name="dz1x", bufs=2) as xp1,
                tc.tile_pool(name="dz1p", bufs=2, space="PSUM") as qp1,
            ):
                for j in range(4):
                    tabv = gtab[0][j][:].rearrange("(t p) f -> p t f", p=128)
                    for r in range(C):
                        xs = xp1.tile([64, max(qrows)], bf16, tag="xs")
                        nc.sync.dma_start(
                            out=xs[:, :qrows[j]],
                            in_=d_xTf[:, r * NP + int(qbase[j]):
                                      r * NP + int(qbase[j]) + qrows[j]],
                        )
                        bb = 0
                        for nb in _dense_chunks(qb[j]):
                            gcol = r * NBLK + qblk_base[j] + bb  # diswf col
                            trow = r * qb[j] + bb  # block-row in table j

                            def wr_tab(evv, tabv=tabv, trow=trow, nb=nb):
                                nc.scalar.dma_start(
                                    out=tabv[:, trow:trow + nb, 0:64],
                                    in_=evv,
                                )
                            dense_chunk(
                                qp1, sp1,
                                xs[:, bb * 128:(bb + nb) * 128],
                                None, t_W1,
                                nb,
                                [(t_diswf[:, gcol:gcol + nb], wr_tab)],
                            )
                            bb += nb

            # ---- interleaved: layer-1 edge + per-quarter layer-2 dense + CC
            gmax = int(g_sj.max())

            def sweep_pre(L, s, qp):
                nb, _q = sweeps[s]
                nf = 64 if L == 0 else OUT
                sl = t_sl1 if L == 0 else t_sl2
                slw = 64 if L == 0 else 32
                sl_flags, _seq = sched[s]
                ps = qp.tile([64, SWMAX * BLK], f32, tag="ps")
                # self-loop injection opens each bank's accumulation group
                for lb in range(nb):
                    b = sweep_base[s] + lb
                    nc.tensor.matmul(
                        out=ps[:nf, lb * BLK:(lb + 1) * BLK],
                        lhsT=sl[:, b * slw:b * slw + nf],
                        rhs=t_eye[:],
                        start=sl_flags[lb][0], stop=sl_flags[lb][1],
                    )
                return ps

            def sweep_chunks(L, s, ps, gp, op_, chunks):
                nf = 64 if L == 0 else OUT
                _slf, seq = sched[s]
                for j in chunks:
                    G = int(g_sj[s, j])
                    if G == 0:
                        continue
                    tb = int(call_base[s, j])
                    gb = gp.tile([128, gmax, 128], bf16, tag="gb")
                    nc.gpsimd.dma_gather(
                        out_ap=gb[:, :G, :],
                        in_ap=gtab[L][j][:, :],
                        idxs_ap=t_idx[:, tb * 8:tb * 8 + G * 8],
                        num_idxs=G * 128,
                        num_idxs_reg=G * 128,
                        elem_size=128,
                        single_packet=False,
                    )
                    todo = seq[j]
                    for g0 in range(0, len(todo), OH_GRP):
                        grp = todo[g0:g0 + OH_GRP]
                        ng = len(grp)
                        oh = op_.tile([128, OH_GRP, BLK], bf16, tag="oh")
                        dl0 = tb + grp[0][0]
                        nc.vector.tensor_tensor(
                            out=oh[:, :ng, :].rearrange(
                                "p g (a b) -> p g a b", b=2),
                            in0=t_iota[:].rearrange("p (a b) -> p a b", b=2)
                                .unsqueeze(1).to_broadcast([128, ng, 64, 2]),
                            in1=t_dlw2[:, 2 * dl0:2 * (dl0 + ng)]
                                .rearrange("p (g b) -> p g b", b=2)
                                .unsqueeze(2).to_broadcast([128, ng, 64, 2]),
                            op=mybir.AluOpType.is_equal,
                        )
                        for k, (cu, lb, fst, lst) in enumerate(grp):
                            nc.tensor.matmul(
                                out=ps[:nf, lb * BLK:(lb + 1) * BLK],
                                lhsT=gb[:, cu, 0:nf],
                                rhs=oh[:, k, :],
                                start=fst, stop=lst,
                            )
            def sweep_fin(L, s, ps, fp, sop):
                nb, _q = sweeps[s]
                bias = t_b1 if L == 0 else t_b2
                nf = 64 if L == 0 else OUT
                if L == 1:
                    ob = sop.tile([OUT, SWMAX * BLK], f32, tag="ob")
                for lb in range(nb):
                    gcol = (sweep_base[s] + lb) * BLK
                    ft = fp.tile([nf, BLK], f32, tag="ft")
                    nc.vector.tensor_tensor(
                        out=ft[:],
                        in0=ps[:nf, lb * BLK:(lb + 1) * BLK],
                        in1=t_dist[:nf, gcol:gcol + BLK],
                        op=mybir.AluOpType.mult,
                    )
                    dst_ap = (t_h1T[:, gcol:gcol + BLK] if L == 0
                              else ob[:, lb * BLK:(lb + 1) * BLK])
                    nc.scalar.activation(
                        out=dst_ap, in_=ft[:],
                        func=mybir.ActivationFunctionType.Relu,
                        bias=bias[:, :1], scale=1.0,
                    )
                if L == 1:
                    c0 = sweep_base[s] * BLK
                    nc.sync.dma_start(
                        out=d_out[:, c0:c0 + nb * BLK],
                        in_=ob[:, :nb * BLK],
                    )

            def edge_sweep(L, s, gp, op_, fp, qp, sop):
                ps = sweep_pre(L, s, qp)
                sweep_chunks(L, s, ps, gp, op_, (0, 1, 2, 3))
                sweep_fin(L, s, ps, fp, sop)

            g2v = g2c[:].rearrange("(t p) f -> p t f", p=128)
            with (
                tc.tile_pool(name="eg0", bufs=4) as gp0,
                tc.tile_pool(name="eo0", bufs=6) as op0,
                tc.tile_pool(name="ef0", bufs=4) as fp0,
                tc.tile_pool(name="ep0", bufs=3, space="PSUM") as qp0,
                tc.tile_pool(name="es0", bufs=2) as sop0,
                tc.tile_pool(name="dz2s", bufs=2) as sp2,
                tc.tile_pool(name="dz2p", bufs=2, space="PSUM") as qp2,
            ):
                for qq in range(4):
                    for s in range(nsw):
                        if sweeps[s][1] == qq:
                            edge_sweep(0, s, gp0, op0, fp0, qp0, sop0)
                    # layer-2 dense for this quarter's own nodes, then CC
                    bb = 0
                    for nb in _dense_chunks(qb[qq]):
                        bglob = qblk_base[qq] + bb

                        def wr_g2(evv, bglob=bglob, nb=nb):
                            nc.sync.dma_start(
                                out=g2v[:, bglob:bglob + nb, :],
                                in_=evv[:, :, 0:32],
                            )

                        def wr_sl2(evv, bglob=bglob, nb=nb):
                            nc.vector.tensor_copy(
                                out=t_sl2[:].rearrange(
                                    "p (t f) -> p t f", f=32)
                                [:, bglob:bglob + nb, :],
                                in_=evv[:, :, 0:32],
                            )
                        dense_chunk(
                            qp2, sp2,
                            t_h1T[:, bglob * 128:(bglob + nb) * 128],
                            None, t_W2p,
                            nb,
                            [(t_disw[:, bglob:bglob + nb], wr_g2),
                             (t_disw[:, bglob:bglob + nb], wr_sl2)],
                        )
                        bb += nb
                    nc.gpsimd.collective_compute(
                        "AllGather", mybir.AluOpType.bypass,
                        replica_groups=[list(range(C))],
                        ins=[g2c[int(qbase[qq]):int(qbase[qq]) + qrows[qq],
                                 :].opt()],
                        outs=[g2cg[qq][:].opt()],
                    )
                    # expand compact [rows,32] into 256B gather rows
                    nc.sync.dma_start(
                        out=gtab[1][qq][:, 0:32],
                        in_=g2cg[qq][:].rearrange("r (a f) -> (r a) f", a=4),
                    )

            # ---- layer-2 edge
            with (
                tc.tile_pool(name="eg1", bufs=4) as gp1,
                tc.tile_pool(name="eo1", bufs=6) as op1,
                tc.tile_pool(name="ef1", bufs=4) as fp1,
                tc.tile_pool(name="ep1", bufs=4, space="PSUM") as qp1b,
                tc.tile_pool(name="es1", bufs=2) as sop1,
            ):
                K = 4
                pss = {}
                for s in range(min(K, nsw)):
                    pss[s] = sweep_pre(1, s, qp1b)
                    sweep_chunks(1, s, pss[s], gp1, op1, (0, 1, 2))
                for s in range(nsw):
                    if s < K:
                        sweep_chunks(1, s, pss[s], gp1, op1, (3,))
                        sweep_fin(1, s, pss.pop(s), fp1, sop1)
                    else:
                        edge_sweep(1, s, gp1, op1, fp1, qp1b, sop1)

    nc.finalize()
    return nc


# ----------------------------------------------------------------------------
# Entry point
# ----------------------------------------------------------------------------

_CACHE = {}


def _prepare(x, edge_index, W1, b1, W2, b2):
    ei = np.asarray(edge_index, dtype=np.int64)
    key = (ei.shape, hash(ei[:, ::65537].tobytes()))
    if _CACHE.get("key") != key:
        meta, per_core, _dis, diswf = _pack(ei)
        nc = _build(meta)
        _CACHE.update(key=key, meta=meta, per_core=per_core, nc=nc,
                      diswf=diswf)
    in_maps = _stage_inputs(x, W1, b1, W2, b2, _CACHE["meta"],
                            _CACHE["per_core"], _CACHE["diswf"])
    return _CACHE["nc"], in_maps


def kernel(x, edge_index, W1, b1, W2, b2):
    from concourse.bass_utils import run_bass_kernel_spmd

    nc, in_maps = _prepare(x, edge_index, W1, b1, W2, b2)
    res = run_bass_kernel_spmd(nc, in_maps, core_ids=list(range(C)))
    outs = []
    for c in range(C):
        outs.append(res.results[c]["outT"][:, :NPC])
    return np.concatenate(outs, axis=1).T.astype(np.float32)


# ----------------------------------------------------------------------------
# Host-side emulation (fast validation of the packing; no HW)
# ----------------------------------------------------------------------------

def emulate(x, edge_index, W1, b1, W2, b2):
    x = np.asarray(x, np.float32)
    meta, per_core, dis, _diswf = _pack(np.asarray(edge_index, np.int64))
    sweeps, sweep_base = meta["sweeps"], meta["sweep_base"]
    qrows = meta["qrows"]
    g_sj, call_base = meta["g_sj"], meta["call_base"]
    sched = _program_schedule(meta)
    W2p = np.concatenate([np.asarray(W2, np.float32),
                          np.zeros((HID, HID - OUT), np.float32)], 1)
    out_full = np.zeros((N, OUT), np.float32)

    def run_layer(acts, W, bias, nf):
        gown = []
        sloop = []
        for c in range(C):
            disp = np.ones(NP, np.float32)
            disp[:NPC] = dis[c * NPC:(c + 1) * NPC]
            g = (acts[c] @ W) * disp[:, None]
            gown.append(g.astype(np.float32))
            sloop.append(g)
        qa = np.cumsum([0] + qrows[:-1])
        gtabs = [np.concatenate([gown[r][qa[j]:qa[j] + qrows[j]]
                                 for r in range(C)]) for j in range(4)]
        new_acts = []
        for c in range(C):
            pc = per_core[c]
            idxw = pc["idxw"]
            dlw2 = pc["dlw2"].astype(np.float32)
            disp = np.ones(NP, np.float32)
            disp[:NPC] = dis[c * NPC:(c + 1) * NPC]
            sT = sloop[c].T[:64].copy()  # self-loop injection
            for s in range(len(sweeps)):
                _slf, seq = sched[s]
                for j in range(4):
                    G = int(g_sj[s, j])
                    if G == 0:
                        continue
                    tb = int(call_base[s, j])
                    iw = idxw[:16, tb * 8:(tb + G) * 8]
                    idxs = iw.T.reshape(-1)
                    rows = gtabs[j][idxs]
                    for (cu, lb, _f, _l) in seq[j]:
                        t = tb + cu
                        msg = rows[cu * 128:(cu + 1) * 128]
                        dl = dlw2[:, 2 * t]
                        oh = (dl[:, None] ==
                              np.arange(BLK, dtype=np.float32)[None, :])
                        blkcol = (sweep_base[s] + lb) * BLK
                        sT[:, blkcol:blkcol + BLK] += msg.T @ oh
            act = np.maximum(sT[:nf] * disp[None, :] + bias.reshape(-1, 1),
                             0.0)
            aT = np.zeros((NP, 64), np.float32)
            aT[:, :nf] = act.T
            new_acts.append(aT)
        return new_acts

    acts = []
    for c in range(C):
        a = np.zeros((NP, 64), np.float32)
        a[:NPC] = x[c * NPC:(c + 1) * NPC]
        acts.append(a)
    acts = run_layer(acts, np.asarray(W1, np.float32),
                     np.asarray(b1, np.float32), 64)
    acts = run_layer(acts, W2p, np.asarray(b2, np.float32), OUT)
    for c in range(C):
        out_full[c * NPC:(c + 1) * NPC] = acts[c][:NPC, :OUT]
    return out_full


# revision 10
# speedup vs baseline: 1.0684x; 1.0322x over previous
"""2-layer GCN (GCNConv x2 + ReLU) on 8 Trainium2 NeuronCores.

Contract: kernel(**inputs) takes FULL inputs (x [100000,64] f32,
edge_index [2,1600000] i32, W1 [64,64], b1 [64], W2 [64,32], b2 [32])
and returns the FULL output [100000, 32] f32.

Strategy (graph/data parallel, by-dst gather, bf16 compute):
  - Nodes sharded 8 ways by contiguous dst range (12500/core, padded to
    12544 = 98 blocks of 128). out = relu(dis * scatter_add_dst(g[src])
    + b) with g = (act @ W) * dis.
  - Self-loops are NOT in the edge stream: per dst block a single
    identity matmul injects sl = (act @ W) * dis^2 into the PSUM
    accumulator (lhsT=sl_block, rhs=I128), also opening each PSUM
    accumulation bank (start flag).
  - All tables/operands bf16 (PE 1 cycle/row vs 4 for fp32; DVE 2x_1p).
    Gather rows are 256B (dma_gather minimum): [64 real | 64 junk] for
    layer 1, [32 real | 96 junk] for layer 2.
  - Layer-1 dense is REPLICATED (full padded graph per core) from a
    host-staged bf16 transposed x; layer-2 dense is own-shard only,
    exported via 4 chunked AllGathers of the compact real-only [*,32]
    bf16 table (4x fewer bytes than padded rows), then expanded into
    256B gather rows with a strided DRAM-to-DRAM copy.
  - A separate tiny own-shard dense pass produces the self-loop tiles
    in SBUF (per-core input; no DRAM round trip).
  - Edge phase: host packs edges into 128-edge tiles grouped by
    (sweep of up to 8 dst blocks, src-quarter chunk, dst block) with
    per-(block,chunk) tile quotas = max over cores so all 8 cores run
    ONE program. dma_gather pulls 256B rows; one-hot built by is_equal
    on bf16 APs whose last dim is a real [1,2] pair (dl staged
    duplicated) to hit the DVE 2x_1p mode; PE matmul msg^T @ onehot
    accumulates feat-major PSUM per block; finalize multiplies dis[dst]
    and applies Relu+bias on the Activation engine.
"""

import sys

if "/opt/trn_rl_repo" not in sys.path:
    sys.path.insert(0, "/opt/trn_rl_repo")

import numpy as np
from ml_dtypes import bfloat16

N = 100000
IN = 64
HID = 64
OUT = 32
C = 8                  # cores
NPC = N // C           # 12500 real nodes per core
BLK = 128              # dst nodes per block / one-hot width
NBLK = 98              # blocks per core (12544 padded nodes)
NP = NBLK * BLK        # 12544 padded nodes per core
SWMAX = 8              # max blocks per sweep (2 PSUM banks)
DCH = 8                # dense-phase blocks per psum chunk (1 bank)
PADDL = 300.0          # dstlocal for pad slots (no one-hot match)
OH_GRP = 8             # tiles per chained one-hot build


def _quarters():
    """Node-quarters (in blocks) per core; chunk j gather table covers
    all 8 ranks' quarter-j rows and must stay < 32767 rows."""
    q = NBLK // 4
    qb = [q, q, q, NBLK - 3 * q]
    assert max(qb) * BLK * C < 32767
    return qb


def _sweeps():
    """[(n_blocks, quarter)] covering each quarter with <=SWMAX blocks."""
    out = []
    for j, nq in enumerate(_quarters()):
        left = nq
        while left > 0:
            take = min(SWMAX, left)
            out.append((take, j))
            left -= take
    return out


# ----------------------------------------------------------------------------
# Host-side packing
# ----------------------------------------------------------------------------

def _pack(edge_index):
    src = edge_index[0].astype(np.int64)
    dst = edge_index[1].astype(np.int64)

    # deg includes self-loops (reference adds them), >= 1
    deg = (np.bincount(dst, minlength=N) + 1).astype(np.float32)
    dis = (1.0 / np.sqrt(deg)).astype(np.float32)

    qb = _quarters()
    qrows = [b * BLK for b in qb]
    qbase = np.cumsum([0] + qrows[:-1])        # row base of quarter, padded
    trows = [C * r for r in qrows]             # gather-table rows per chunk

    # gather-table position of a source node (within its chunk's table)
    rank = src // NPC
    off = src % NPC
    chunk = np.searchsorted(qbase, off, side="right") - 1   # 0..3
    tidx = rank * np.asarray(qrows)[chunk] + (off - qbase[chunk])

    core = dst // NPC
    dloc = dst - core * NPC
    block = dloc // BLK
    dlb = dloc % BLK

    key = (core * NBLK + block) * 4 + chunk
    counts = np.bincount(key, minlength=C * NBLK * 4).reshape(C, NBLK, 4)
    quota = -(-counts.max(axis=0) // 128)  # [NBLK, 4]

    sweeps = _sweeps()
    nsw = len(sweeps)
    szs = [s[0] for s in sweeps]
    sweep_base = np.cumsum([0] + szs[:-1])
    sweep_of_block = np.repeat(np.arange(nsw), szs)
    lb_of_block = np.arange(NBLK) - sweep_base[sweep_of_block]

    sweep_goff = np.cumsum([0] + [4 * sz for sz in szs[:-1]])
    gid_of_bj = (sweep_goff[sweep_of_block][:, None]
                 + np.arange(4)[None, :] * np.array(szs)[sweep_of_block][:, None]
                 + lb_of_block[:, None])
    ngroups = 4 * NBLK
    gq = np.zeros(ngroups, np.int64)
    gq[gid_of_bj.reshape(-1)] = quota.reshape(-1)
    gbase = np.zeros_like(gq)
    np.cumsum(gq[:-1], out=gbase[1:])
    tiles_total = int(gq.sum())
    slots_total = tiles_total * 128

    g_sj = np.zeros((nsw, 4), np.int64)
    call_base = np.zeros((nsw, 4), np.int64)
    for s in range(nsw):
        b0 = sweep_base[s]
        for j in range(4):
            g_sj[s, j] = quota[b0:b0 + szs[s], j].sum()
    cb = np.zeros(nsw * 4, np.int64)
    np.cumsum(g_sj.reshape(-1)[:-1], out=cb[1:])
    call_base[:] = cb.reshape(nsw, 4)

    meta = dict(quota=quota, sweeps=sweeps, sweep_base=sweep_base,
                qb=qb, qrows=qrows, qbase=qbase, trows=trows,
                g_sj=g_sj, call_base=call_base, tiles_total=tiles_total,
                slots_total=slots_total)

    per_core = []
    for c in range(C):
        m = core == c
        gid = gid_of_bj[block[m], chunk[m]]
        order = np.argsort(gid, kind="stable")
        gid_s = gid[order]
        grp_start = np.searchsorted(gid_s, np.arange(ngroups))
        pos = np.arange(gid_s.size) - grp_start[gid_s]
        slot = gbase[gid_s] * 128 + pos
        assert (pos < gq[gid_s] * 128).all()

        idx_slots = np.zeros(slots_total, np.int16)
        dl_slots = np.full(slots_total, PADDL, np.float32)
        idx_slots[slot] = tidx[m][order].astype(np.int16)
        dl_slots[slot] = dlb[m][order].astype(np.float32)

        iw = idx_slots.reshape(-1, 16).T.copy()
        idxw = np.tile(iw, (8, 1))
        # dl duplicated x2 along a trailing dim so the one-hot in1 AP ends
        # with a real [1,2] packed pair (DVE 2x_1p requirement)
        dlt = dl_slots.reshape(-1, 128).T.astype(bfloat16)   # [128, tiles]
        dlw2 = np.repeat(dlt, 2, axis=1)                     # [128, 2*tiles]

        dis_own = np.ones(NP, np.float32)
        dis_own[:NPC] = dis[c * NPC:(c + 1) * NPC]
        disw = dis_own.reshape(NBLK, 128).T.astype(bfloat16)   # [128, NBLK]
        disqw = (dis_own * dis_own).reshape(NBLK, 128).T.astype(bfloat16)
        dist = np.tile(dis_own[None, :], (64, 1)).astype(bfloat16)  # [64, NP]

        per_core.append(dict(idxw=idxw, dlw2=dlw2, disw=disw, disqw=disqw,
                             dist=dist))

    # replicated-dense dis staging (same for all cores)
    dis_pad_full = np.ones(C * NP, np.float32)
    for c in range(C):
        dis_pad_full[c * NP:c * NP + NPC] = dis[c * NPC:(c + 1) * NPC]
    diswf = dis_pad_full.reshape(C * NBLK, 128).T.astype(bfloat16)

    return meta, per_core, dis, diswf


def _stage_inputs(x, W1, b1, W2, b2, meta, per_core, diswf):
    x = np.asarray(x, np.float32)
    W2p = np.concatenate([np.asarray(W2, np.float32),
                          np.zeros((HID, HID - OUT), np.float32)], axis=1)
    iota = np.tile(np.arange(BLK, dtype=np.float32), (128, 1)).astype(bfloat16)
    eye = np.eye(128, dtype=np.float32).astype(bfloat16)
    xTf = np.zeros((IN, C * NP), np.float32)
    for r in range(C):
        xTf[:, r * NP:r * NP + NPC] = x[r * NPC:(r + 1) * NPC].T
    xTf = xTf.astype(bfloat16)
    in_maps = []
    for c in range(C):
        pc = per_core[c]
        in_maps.append({
            "xTf": xTf,
            "xTo": xTf[:, c * NP:(c + 1) * NP].copy(),
            "diswf": diswf,
            "disw": pc["disw"],
            "disqw": pc["disqw"],
            "dist": pc["dist"],
            "idxw": pc["idxw"],
            "dlw2": pc["dlw2"],
            "iota": iota,
            "eye": eye,
            "W1": np.asarray(W1, np.float32).astype(bfloat16),
            "W2p": W2p.astype(bfloat16),
            "b1": np.asarray(b1, np.float32).reshape(HID, 1),
            "b2": np.asarray(b2, np.float32).reshape(OUT, 1),
        })
    return in_maps


def _program_schedule(meta):
    """Per sweep: (sl_flags, seq). sl_flags[lb] = (start, stop) for the
    self-loop matmul of local block lb (emitted FIRST, before all edge
    matmuls). seq[j] = [(cursor_in_call, local_block, start, stop)].
    Flags are at per-(sweep, psum-bank) granularity; the self-loop
    matmul of the first block in each bank always carries start."""
    quota, sweeps, sweep_base = meta["quota"], meta["sweeps"], meta["sweep_base"]
    sched = []
    for s, (nb, _q) in enumerate(sweeps):
        b0 = sweep_base[s]
        seq = []
        for j in range(4):
            cur = 0
            call = []
            for lb in range(nb):
                q = int(quota[b0 + lb, j])
                for r in range(q):
                    call.append([cur, lb, False, False])
                    cur += 1
            seq.append(call)
        sl_flags = [[False, False] for _ in range(nb)]
        nbank = (nb + 3) // 4
        for k in range(nbank):
            # first touch: self-loop matmul of the bank's first block
            sl_flags[4 * k][0] = True
            # last touch: last edge matmul in this bank, else last self-loop
            touch = [(j, i) for j in range(4) for i, e in enumerate(seq[j])
                     if e[1] // 4 == k]
            if touch:
                j1, i1 = touch[-1]
                seq[j1][i1][3] = True
            else:
                lb_last = min(4 * k + 3, nb - 1)
                sl_flags[lb_last][1] = True
        sched.append((sl_flags, seq))
    return sched


def _dense_chunks(nblocks):
    out = []
    left = nblocks
    while left > 0:
        out.append(min(DCH, left))
        left -= out[-1]
    return out


# ----------------------------------------------------------------------------
# Device program (identical on all 8 cores)
# ----------------------------------------------------------------------------

def _build(meta):
    from concourse import bacc, mybir, tile

    sweeps = meta["sweeps"]
    nsw = len(sweeps)
    sweep_base = meta["sweep_base"]
    qb, qrows, qbase, trows = (meta["qb"], meta["qrows"], meta["qbase"],
                               meta["trows"])
    g_sj = meta["g_sj"]
    call_base = meta["call_base"]
    tiles_total = meta["tiles_total"]
    slots_total = meta["slots_total"]
    sched = _program_schedule(meta)
    qblk_base = [int(b) // BLK for b in qbase]   # quarter base, in blocks
    f32 = mybir.dt.float32
    bf16 = mybir.dt.bfloat16

    nc = bacc.Bacc(num_devices=C)
    d_xTf = nc.dram_tensor("xTf", [IN, C * NP], bf16, kind="ExternalInput")
    d_xTo = nc.dram_tensor("xTo", [IN, NP], bf16, kind="ExternalInput")
    d_diswf = nc.dram_tensor("diswf", [128, C * NBLK], bf16,
                             kind="ExternalInput")
    d_disw = nc.dram_tensor("disw", [128, NBLK], bf16, kind="ExternalInput")
    d_disqw = nc.dram_tensor("disqw", [128, NBLK], bf16, kind="ExternalInput")
    d_dist = nc.dram_tensor("dist", [64, NP], bf16, kind="ExternalInput")
    d_idxw = nc.dram_tensor("idxw", [128, slots_total // 16], mybir.dt.int16,
                            kind="ExternalInput")
    d_dlw2 = nc.dram_tensor("dlw2", [128, 2 * tiles_total], bf16,
                            kind="ExternalInput")
    d_iota = nc.dram_tensor("iota", [128, BLK], bf16, kind="ExternalInput")
    d_eye = nc.dram_tensor("eye", [128, 128], bf16, kind="ExternalInput")
    d_W1 = nc.dram_tensor("W1", [IN, HID], bf16, kind="ExternalInput")
    d_W2p = nc.dram_tensor("W2p", [HID, HID], bf16, kind="ExternalInput")
    d_b1 = nc.dram_tensor("b1", [HID, 1], f32, kind="ExternalInput")
    d_b2 = nc.dram_tensor("b2", [OUT, 1], f32, kind="ExternalInput")
    d_out = nc.dram_tensor("outT", [OUT, NP], f32, kind="ExternalOutput")

    with tile.TileContext(nc) as tc:
        with (
            tc.tile_pool(name="persist", bufs=1) as pp,
            tc.tile_pool(name="dram", bufs=1, space="DRAM") as dp,
        ):
            t_dlw2 = pp.tile([128, 2 * tiles_total], bf16, tag="dlw2")
            t_idx = pp.tile([128, slots_total // 16], mybir.dt.int16,
                            tag="idx")
            t_iota = pp.tile([128, BLK], bf16, tag="iota")
            t_eye = pp.tile([128, 128], bf16, tag="eye")
            t_W1 = pp.tile([IN, HID], bf16, tag="W1")
            t_W2p = pp.tile([HID, HID], bf16, tag="W2p")
            t_b1 = pp.tile([HID, 1], f32, tag="b1")
            t_b2 = pp.tile([OUT, 1], f32, tag="b2")
            t_diswf = pp.tile([128, C * NBLK], bf16, tag="diswf")
            t_disw = pp.tile([128, NBLK], bf16, tag="disw")
            t_disqw = pp.tile([128, NBLK], bf16, tag="disqw")
            t_dist = pp.tile([64, NP], bf16, tag="dist")
            t_h1T = pp.tile([64, NP], bf16, tag="h1T")
            t_sl1 = pp.tile([128, NBLK * 64], bf16, tag="sl1")
            t_sl2 = pp.tile([128, NBLK * 32], bf16, tag="sl2")

            # dense-phase prerequisites first; bulky edge-phase-only
            # tensors (idx/dl/dist) are deferred below so the dense
            # pipeline starts ~20us earlier
            nc.sync.dma_start(out=t_W1[:], in_=d_W1[:])
            nc.sync.dma_start(out=t_W2p[:], in_=d_W2p[:])
            nc.sync.dma_start(out=t_b1[:], in_=d_b1[:])
            nc.sync.dma_start(out=t_b2[:], in_=d_b2[:])
            nc.sync.dma_start(out=t_diswf[:], in_=d_diswf[:])
            nc.sync.dma_start(out=t_disw[:], in_=d_disw[:])
            nc.sync.dma_start(out=t_disqw[:], in_=d_disqw[:])
            nc.sync.dma_start(out=t_eye[:], in_=d_eye[:])

            # DRAM scratch: gather tables (256B rows) + compact L2 export
            gtab = [[dp.tile([trows[j], 128], bf16, name=f"gtab{L}_{j}",
                             tag=f"gtab{L}_{j}")
                     for j in range(4)] for L in range(2)]
            g2c = dp.tile([NP, 32], bf16, name="g2c", tag="g2c")
            g2cg = [dp.tile([trows[j] // 4, 128], bf16, name=f"g2cg{j}",
                            tag=f"g2cg{j}")
                    for j in range(4)]

            def dense_chunk(qp, sp, lhs_ap, scale_cols, W, nb, outs):
                """One psum chunk: nb block-matmuls, then for each
                (scale_col_ap, dst writer) in outs: evict psum*scale."""
                p = qp.tile([128, DCH * 64], f32, tag="p")
                for t in range(nb):
                    nc.tensor.matmul(
                        out=p[:, t * 64:(t + 1) * 64],
                        lhsT=lhs_ap[:, t * 128:(t + 1) * 128],
                        rhs=W[:],
                        start=(t == 0), stop=(t == nb - 1),
                    )
                pv = p[:].rearrange("p (t f) -> p t f", f=64)[:, :nb, :]
                for scale_ap, writer in outs:
                    ev = sp.tile([128, DCH * 64], bf16, tag="ev")
                    evv = ev[:].rearrange("p (t f) -> p t f", f=64)[:, :nb, :]
                    nc.vector.tensor_tensor(
                        out=evv, in0=pv,
                        in1=scale_ap.unsqueeze(2).to_broadcast([128, nb, 64]),
                        op=mybir.AluOpType.mult,
                    )
                    writer(evv)
                _ = scale_cols  # unused; kept for call-site clarity

            # ---- own-shard dense passes -> self-loop tiles in SBUF
            # (tiny: no DRAM traffic; per-core via xTo/disqw inputs)
            with (
                tc.tile_pool(name="dzos", bufs=2) as spo,
                tc.tile_pool(name="dzox", bufs=2) as xpo,
                tc.tile_pool(name="dzop", bufs=2, space="PSUM") as qpo,
            ):
                xo = xpo.tile([64, NP], bf16, tag="xo")
                nc.sync.dma_start(out=xo[:], in_=d_xTo[:])
                bb = 0
                for nb in _dense_chunks(NBLK):
                    def wr_sl1(evv, bb=bb, nb=nb):
                        nc.vector.tensor_copy(
                            out=t_sl1[:].rearrange("p (t f) -> p t f", f=64)
                            [:, bb:bb + nb, :],
                            in_=evv,
                        )
                    dense_chunk(
                        qpo, spo, xo[:, bb * 128:(bb + nb) * 128],
                        None, t_W1,
                        nb,
                        [(t_disw[:, bb:bb + nb], wr_sl1)],
                    )
                    bb += nb

            nc.sync.dma_start(out=t_dlw2[:], in_=d_dlw2[:])
            nc.sync.dma_start(out=t_idx[:], in_=d_idxw[:])
            nc.sync.dma_start(out=t_iota[:], in_=d_iota[:])
            nc.sync.dma_start(out=t_dist[:], in_=d_dist[:])

            # ---- layer-1 dense, replicated over the full padded graph.
            # quarter-major so gather table j completes early.
            with (
                tc.tile_pool(name="dz1s", bufs=3) as sp1,
                tc.tile_pool(name="dz1x", bufs=2) as xp1,
                tc.tile_pool(name="dz1p", bufs=2, space="PSUM") as qp1,
            ):
                for j in range(4):
                    tabv = gtab[0][j][:].rearrange("(t p) f -> p t f", p=128)
                    for r in range(C):
                        xs = xp1.tile([64, max(qrows)], bf16, tag="xs")
                        nc.sync.dma_start(
                            out=xs[:, :qrows[j]],
                            in_=d_xTf[:, r * NP + int(qbase[j]):
                                      r * NP + int(qbase[j]) + qrows[j]],
                        )
                        bb = 0
                        for nb in _dense_chunks(qb[j]):
                            gcol = r * NBLK + qblk_base[j] + bb  # diswf col
                            trow = r * qb[j] + bb  # block-row in table j

                            def wr_tab(evv, tabv=tabv, trow=trow, nb=nb):
                                nc.scalar.dma_start(
                                    out=tabv[:, trow:trow + nb, 0:64],
                                    in_=evv,
                                )
                            dense_chunk(
                                qp1, sp1,
                                xs[:, bb * 128:(bb + nb) * 128],
                                None, t_W1,
                                nb,
                                [(t_diswf[:, gcol:gcol + nb], wr_tab)],
                            )
                            bb += nb

            # ---- interleaved: layer-1 edge + per-quarter layer-2 dense + CC
            gmax = int(g_sj.max())

            def sweep_pre(L, s, qp):
                nb, _q = sweeps[s]
                nf = 64 if L == 0 else OUT
                sl = t_sl1 if L == 0 else t_sl2
                slw = 64 if L == 0 else 32
                sl_flags, _seq = sched[s]
                ps = qp.tile([64, SWMAX * BLK], f32, tag="ps")
                # self-loop injection opens each bank's accumulation group
                for lb in range(nb):
                    b = sweep_base[s] + lb
                    nc.tensor.matmul(
                        out=ps[:nf, lb * BLK:(lb + 1) * BLK],
                        lhsT=sl[:, b * slw:b * slw + nf],
                        rhs=t_eye[:],
                        start=sl_flags[lb][0], stop=sl_flags[lb][1],
                    )
                return ps

            def sweep_chunks(L, s, ps, gp, op_, chunks):
                nf = 64 if L == 0 else OUT
                _slf, seq = sched[s]
                for j in chunks:
                    G = int(g_sj[s, j])
                    if G == 0:
                        continue
                    tb = int(call_base[s, j])
                    gb = gp.tile([128, gmax, 128], bf16, tag="gb")
                    nc.gpsimd.dma_gather(
                        out_ap=gb[:, :G, :],
                        in_ap=gtab[L][j][:, :],
                        idxs_ap=t_idx[:, tb * 8:tb * 8 + G * 8],
                        num_idxs=G * 128,
                        num_idxs_reg=G * 128,
                        elem_size=128,
                        single_packet=False,
                    )
                    todo = seq[j]
                    for g0 in range(0, len(todo), OH_GRP):
                        grp = todo[g0:g0 + OH_GRP]
                        ng = len(grp)
                        oh = op_.tile([128, OH_GRP, BLK], bf16, tag="oh")
                        dl0 = tb + grp[0][0]
                        nc.vector.tensor_tensor(
                            out=oh[:, :ng, :].rearrange(
                                "p g (a b) -> p g a b", b=2),
                            in0=t_iota[:].rearrange("p (a b) -> p a b", b=2)
                                .unsqueeze(1).to_broadcast([128, ng, 64, 2]),
                            in1=t_dlw2[:, 2 * dl0:2 * (dl0 + ng)]
                                .rearrange("p (g b) -> p g b", b=2)
                                .unsqueeze(2).to_broadcast([128, ng, 64, 2]),
                            op=mybir.AluOpType.is_equal,
                        )
                        for k, (cu, lb, fst, lst) in enumerate(grp):
                            nc.tensor.matmul(
                                out=ps[:nf, lb * BLK:(lb + 1) * BLK],
                                lhsT=gb[:, cu, 0:nf],
                                rhs=oh[:, k, :],
                                start=fst, stop=lst,
                            )
            def sweep_fin(L, s, ps, fp, sop):
                nb, _q = sweeps[s]
                bias = t_b1 if L == 0 else t_b2
                nf = 64 if L == 0 else OUT
                if L == 1:
                    ob = sop.tile([OUT, SWMAX * BLK], f32, tag="ob")
                for lb in range(nb):
                    gcol = (sweep_base[s] + lb) * BLK
                    ft = fp.tile([nf, BLK], f32, tag="ft")
                    nc.vector.tensor_tensor(
                        out=ft[:],
                        in0=ps[:nf, lb * BLK:(lb + 1) * BLK],
                        in1=t_dist[:nf, gcol:gcol + BLK],
                        op=mybir.AluOpType.mult,
                    )
                    dst_ap = (t_h1T[:, gcol:gcol + BLK] if L == 0
                              else ob[:, lb * BLK:(lb + 1) * BLK])
                    nc.scalar.activation(
                        out=dst_ap, in_=ft[:],
                        func=mybir.ActivationFunctionType.Relu,
                        bias=bias[:, :1], scale=1.0,
                    )
                if L == 1:
                    c0 = sweep_base[s] * BLK
                    nc.sync.dma_start(
                        out=d_out[:, c0:c0 + nb * BLK],
                        in_=ob[:, :nb * BLK],
                    )

            def edge_sweep(L, s, gp, op_, fp, qp, sop):
                ps = sweep_pre(L, s, qp)
                sweep_chunks(L, s, ps, gp, op_, (0, 1, 2, 3))
                sweep_fin(L, s, ps, fp, sop)

            g2v = g2c[:].rearrange("(t p) f -> p t f", p=128)
            with (
                tc.tile_pool(name="eg0", bufs=4) as gp0,
                tc.tile_pool(name="eo0", bufs=6) as op0,
                tc.tile_pool(name="ef0", bufs=4) as fp0,
                tc.tile_pool(name="ep0", bufs=3, space="PSUM") as qp0,
                tc.tile_pool(name="es0", bufs=2) as sop0,
                tc.tile_pool(name="dz2s", bufs=2) as sp2,
                tc.tile_pool(name="dz2p", bufs=2, space="PSUM") as qp2,
            ):
                for qq in range(4):
                    for s in range(nsw):
                        if sweeps[s][1] == qq:
                            edge_sweep(0, s, gp0, op0, fp0, qp0, sop0)
                    # layer-2 dense for this quarter's own nodes, then CC
                    bb = 0
                    for nb in _dense_chunks(qb[qq]):
                        bglob = qblk_base[qq] + bb

                        def wr_g2(evv, bglob=bglob, nb=nb):
                            nc.sync.dma_start(
                                out=g2v[:, bglob:bglob + nb, :],
                                in_=evv[:, :, 0:32],
                            )

                        def wr_sl2(evv, bglob=bglob, nb=nb):
                            nc.vector.tensor_copy(
                                out=t_sl2[:].rearrange(
                                    "p (t f) -> p t f", f=32)
                                [:, bglob:bglob + nb, :],
                                in_=evv[:, :, 0:32],
                            )
                        dense_chunk(
                            qp2, sp2,
                            t_h1T[:, bglob * 128:(bglob + nb) * 128],
                            None, t_W2p,
                            nb,
                            [(t_disw[:, bglob:bglob + nb], wr_g2),
                             (t_disw[:, bglob:bglob + nb], wr_sl2)],
                        )
                        bb += nb
                    nc.gpsimd.collective_compute(
                        "AllGather", mybir.AluOpType.bypass,
                        replica_groups=[list(range(C))],
                        ins=[g2c[int(qbase[qq]):int(qbase[qq]) + qrows[qq],
                                 :].opt()],
                        outs=[g2cg[qq][:].opt()],
                    )
                    # expand compact [rows,32] into 256B gather rows
                    nc.sync.dma_start(
                        out=gtab[1][qq][:, 0:32],
                        in_=g2cg[qq][:].rearrange("r (a f) -> (r a) f", a=4),
                    )

            # ---- layer-2 edge
            with (
                tc.tile_pool(name="eg1", bufs=4) as gp1,
                tc.tile_pool(name="eo1", bufs=6) as op1,
                tc.tile_pool(name="ef1", bufs=4) as fp1,
                tc.tile_pool(name="ep1", bufs=4, space="PSUM") as qp1b,
                tc.tile_pool(name="es1", bufs=2) as sop1,
            ):
                K = 4
                pss = {}
                for s in range(min(K, nsw)):
                    pss[s] = sweep_pre(1, s, qp1b)
                    sweep_chunks(1, s, pss[s], gp1, op1, (0, 1, 2))
                for s in range(nsw):
                    if s < K:
                        sweep_chunks(1, s, pss[s], gp1, op1, (3,))
                        sweep_fin(1, s, pss.pop(s), fp1, sop1)
                    else:
                        edge_sweep(1, s, gp1, op1, fp1, qp1b, sop1)

    nc.finalize()
    return nc


# ----------------------------------------------------------------------------
# Entry point
# ----------------------------------------------------------------------------

_CACHE = {}


def _prepare(x, edge_index, W1, b1, W2, b2):
    ei = np.asarray(edge_index, dtype=np.int64)
    key = (ei.shape, hash(ei[:, ::65537].tobytes()))
    if _CACHE.get("key") != key:
        meta, per_core, _dis, diswf = _pack(ei)
        nc = _build(meta)
        _CACHE.update(key=key, meta=meta, per_core=per_core, nc=nc,
                      diswf=diswf)
    in_maps = _stage_inputs(x, W1, b1, W2, b2, _CACHE["meta"],
                            _CACHE["per_core"], _CACHE["diswf"])
    return _CACHE["nc"], in_maps


def kernel(x, edge_index, W1, b1, W2, b2):
    from concourse.bass_utils import run_bass_kernel_spmd

    nc, in_maps = _prepare(x, edge_index, W1, b1, W2, b2)
    res = run_bass_kernel_spmd(nc, in_maps, core_ids=list(range(C)))
    outs = []
    for c in range(C):
        outs.append(res.results[c]["outT"][:, :NPC])
    return np.concatenate(outs, axis=1).T.astype(np.float32)


# ----------------------------------------------------------------------------
# Host-side emulation (fast validation of the packing; no HW)
# ----------------------------------------------------------------------------

def emulate(x, edge_index, W1, b1, W2, b2):
    x = np.asarray(x, np.float32)
    meta, per_core, dis, _diswf = _pack(np.asarray(edge_index, np.int64))
    sweeps, sweep_base = meta["sweeps"], meta["sweep_base"]
    qrows = meta["qrows"]
    g_sj, call_base = meta["g_sj"], meta["call_base"]
    sched = _program_schedule(meta)
    W2p = np.concatenate([np.asarray(W2, np.float32),
                          np.zeros((HID, HID - OUT), np.float32)], 1)
    out_full = np.zeros((N, OUT), np.float32)

    def run_layer(acts, W, bias, nf):
        gown = []
        sloop = []
        for c in range(C):
            disp = np.ones(NP, np.float32)
            disp[:NPC] = dis[c * NPC:(c + 1) * NPC]
            g = (acts[c] @ W) * disp[:, None]
            gown.append(g.astype(np.float32))
            sloop.append(g)
        qa = np.cumsum([0] + qrows[:-1])
        gtabs = [np.concatenate([gown[r][qa[j]:qa[j] + qrows[j]]
                                 for r in range(C)]) for j in range(4)]
        new_acts = []
        for c in range(C):
            pc = per_core[c]
            idxw = pc["idxw"]
            dlw2 = pc["dlw2"].astype(np.float32)
            disp = np.ones(NP, np.float32)
            disp[:NPC] = dis[c * NPC:(c + 1) * NPC]
            sT = sloop[c].T[:64].copy()  # self-loop injection
            for s in range(len(sweeps)):
                _slf, seq = sched[s]
                for j in range(4):
                    G = int(g_sj[s, j])
                    if G == 0:
                        continue
                    tb = int(call_base[s, j])
                    iw = idxw[:16, tb * 8:(tb + G) * 8]
                    idxs = iw.T.reshape(-1)
                    rows = gtabs[j][idxs]
                    for (cu, lb, _f, _l) in seq[j]:
                        t = tb + cu
                        msg = rows[cu * 128:(cu + 1) * 128]
                        dl = dlw2[:, 2 * t]
                        oh = (dl[:, None] ==
                              np.arange(BLK, dtype=np.float32)[None, :])
                        blkcol = (sweep_base[s] + lb) * BLK
                        sT[:, blkcol:blkcol + BLK] += msg.T @ oh
            act = np.maximum(sT[:nf] * disp[None, :] + bias.reshape(-1, 1),
                             0.0)
            aT = np.zeros((NP, 64), np.float32)
            aT[:, :nf] = act.T
            new_acts.append(aT)
        return new_acts

    acts = []
    for c in range(C):
        a = np.zeros((NP, 64), np.float32)
        a[:NPC] = x[c * NPC:(c + 1) * NPC]
        acts.append(a)
    acts = run_layer(acts, np.asarray(W1, np.float32),
                     np.asarray(b1, np.float32), 64)
    acts = run_layer(acts, W2p, np.asarray(b2, np.float32), OUT)
    for c in range(C):
        out_full[c * NPC:(c + 1) * NPC] = acts[c][:NPC, :OUT]
    return out_full
